# revision 51
# baseline (speedup 1.0000x reference)
"""Trainium2 Bass kernel for relative-position multi-head attention.

Shapes (hardcoded): B=2, L=384, D=256, H=8, DH=32.
Sharding: 8 cores; core c handles batch b=c//4, query rows [(c%4)*96, +96).
Pure data-parallel SPMD - no collectives.

Math (per batch b, query q):
  q/k/v projections: x @ W.T + bias
  A_C[h,k] = (q_h+u_h) . k_h[k]
  B_D[h,k] = (q_h+v_h) . (Wr_h @ pos[q,k] + br_h)
           = (Wr_h^T (q_h+v_h)) . pos[q,k]   + const(h,q)   [br term is
             k-independent -> cancels in softmax -> dropped]
  score    = (A_C + B_D)/sqrt(DH) - (1-mask[k])*1e15
  out      = softmax_k(score) @ v

Key restructurings for the hardware (v2):
  * r = pos @ Wr.T (38 GFLOP) is never materialized; instead
    T[q] = Wr^T-blockdiag @ (q+v)  (a [256,8] matrix per query) and
    B_D = T^T @ posT  (1.2 GFLOP).
  * pos stays fp32 end-to-end on the DMA/cast path: the PE transposes it
    chunk-wise (float32r transpose mode), and the PSUM->SBUF copy does
    the fp32->bf16 cast on DVE/ACT.  The gpsimd engine (the v1
    bottleneck: 250us of fp32->bf16 casts) does almost nothing.
  * scores live in PSUM as [(16q x 8h) partitions, 384 k free]; per
    16-query group one psum tile gets: 2 batched A_C matmuls (block-
    diagonal (q+u) weights vs k_projT, moving N=384) + per-query B_D
    matmuls (T stationary 32 cols, posT moving N=384).
  * softmax over k (free dim): one exp per group on ACT; the key mask
    and the softmax denominator are folded into an augmented, mask-
    scaled value matrix so normalization falls out of the output matmul.
"""

import sys

for _p in ("/opt/trn_rl_repo", "/root/.axon_site/_ro/trn_rl_repo"):
    if _p not in sys.path:
        sys.path.append(_p)

import numpy as np

import concourse.bass as bass
import concourse.mybir as mybir
import concourse.tile as tile
from concourse import bacc
from concourse.masks import make_identity

FP32 = mybir.dt.float32
FP32R = mybir.dt.float32r
BF16 = mybir.dt.bfloat16

B, L, D, H = 2, 384, 256, 8
DH = D // H            # 32
Q = 96                 # queries per core
KT = L // 128          # 3 k-tiles
CB = D // 128          # 2 contraction blocks
NCORES = 8
PG = 4                 # pairs per DMA batch
NG = Q // 16           # score groups of 16 queries
SCALE = 1.0 / np.sqrt(DH)


def build_kernel_body(tc, outs, ins):
    """Emit the per-core program. outs/ins are dicts of DRAM APs."""
    from contextlib import ExitStack
    ctx = ExitStack()
    pool = lambda **kw: ctx.enter_context(tc.tile_pool(**kw))
    nc = tc.nc
    pos = ins["pos"]          # [Q, L, D] f32
    key = ins["key"]          # [L, D]
    value = ins["value"]      # [L, D]
    query = ins["query"]      # [Q, D]
    mask = ins["mask"]        # [L]
    Wk, Wq, Wv, Wr = ins["Wk"], ins["Wq"], ins["Wv"], ins["Wr"]   # [D, D]
    bk, bq, bv = ins["bk"], ins["bq"], ins["bv"]                  # [D]
    u_in, v_in = ins["u"], ins["v"]                               # [H, DH]
    out = outs["out"]         # [Q, D] f32

    const = pool(name="const", bufs=1)
    setup = pool(name="setup", bufs=2)
    psum_big = pool(name="psum_big", bufs=2, space="PSUM")
    psum_sc = pool(name="psum_sc", bufs=2, space="PSUM")
    psum_posT = pool(name="psum_posT", bufs=3, space="PSUM")
    pair_pool = pool(name="pair", bufs=3)
    pt_pool = pool(name="pt", bufs=4)

    # ---------------- identities ----------------
    ident_f = const.tile([128, 128], FP32)
    make_identity(nc, ident_f)
    ident_b = const.tile([128, 128], BF16)
    nc.vector.tensor_copy(out=ident_b, in_=ident_f)

    # ---------------- load weights + inputs ----------------
    # One consolidated DMA per tensor; T32's dependency chain (Wq -> q-proj
    # -> qv -> T) is loaded first.  Small column loads go on the ACT HWDGE
    # queue so the Pool queue is free to start emitting pos slab DMAs.
    def load_fold(ap, rows, tg, eng=nc.sync):  # [rows, D] dram -> [128, rows//128, D]
        n = rows // 128
        t = setup.tile([128, n, D], FP32, tag=f"ld_{tg}", name=f"ld_{tg}")
        eng.dma_start(out=t, in_=ap.rearrange("(i p) c -> p i c", p=128))
        return [t[:, i, :] for i in range(n)]

    key_n = load_fold(key, L, "key")
    qry_n = setup.tile([96, D], FP32)
    nc.sync.dma_start(out=qry_n, in_=query)
    Wq_n = load_fold(Wq, D, "wq")
    # Wr as [32 dh, 8 h, 256]: per-head lhsT slices at partition base 0
    wr_t = const.tile([DH, H, D], FP32)
    nc.sync.dma_start(
        out=wr_t, in_=Wr.rearrange("(h dh) c -> dh h c", dh=DH))
    Wr_h = [wr_t[:, h, :] for h in range(H)]
    # (slab DMA gate is emitted below, after the last setup DMA)

    def col_load(ap1d, n, tag):  # [n] dram -> list of [128,1] sbuf columns
        t = const.tile([128, n // 128], FP32, tag=f"col_{tag}",
                       name=f"col_{tag}")
        nc.scalar.dma_start(
            out=t, in_=ap1d.rearrange("(i p) -> p i", p=128))
        return [t[:, i:i + 1] for i in range(n // 128)]

    bq_c = col_load(bq, D, "bq")
    u_c = col_load(u_in.rearrange("h d -> (h d)"), D, "u")
    v_c = col_load(v_in.rearrange("h d -> (h d)"), D, "v")
    bk_c = col_load(bk, D, "bk")
    bv_row = const.tile([1, D], FP32)
    nc.scalar.dma_start(out=bv_row, in_=bv.rearrange("(o d) -> o d", o=1))
    # mask columns in permuted order: mask_p[r, j] = mask[3r + j]
    mask_p = const.tile([128, KT], FP32)
    nc.scalar.dma_start(
        out=mask_p, in_=mask.rearrange("(r j) -> r j", j=KT))
    # value path loads last: v_aug is only needed by the output stage
    Wk_n = load_fold(Wk, D, "wk", eng=nc.scalar)
    val_n = load_fold(value, L, "val", eng=nc.scalar)
    Wv_n = load_fold(Wv, D, "wv", eng=nc.scalar)


    # ---------------- transpose helper (fp32, PE) ----------------
    def transpose_to(dst_tiles, src_tiles, rows, cols, tag):
        """src: list of sbuf tiles [<=128, cols] covering [rows, cols].
        dst_tiles: list of CB sbuf tiles [128, rows] covering [cols, rows]."""
        for cb in range(cols // 128):
            ps = psum_big.tile([128, 512], FP32, tag="big", name="ps_tp")
            nrt = len(src_tiles)
            for i, st in enumerate(src_tiles):
                r = st.shape[0]
                nc.tensor.matmul(
                    ps[:, i * 128:i * 128 + r],
                    st[:, cb * 128:(cb + 1) * 128],
                    ident_f[:r, :r],
                    is_transpose=True,
                    start=(i == 0), stop=(i == nrt - 1))
            nc.vector.tensor_copy(out=dst_tiles[cb], in_=ps[:, :rows])

    qryT = [setup.tile([128, Q], FP32, tag=f"qryT{i}", name=f"qryT{i}") for i in range(CB)]
    transpose_to(qryT, [qry_n], Q, D, "q")
    WqT = [setup.tile([128, D], FP32, tag=f"WqT{i}", name=f"WqT{i}") for i in range(CB)]
    transpose_to(WqT, Wq_n, D, D, "wq")
    keyT = [setup.tile([128, L], FP32, tag=f"keyT{i}", name=f"keyT{i}") for i in range(CB)]
    transpose_to(keyT, key_n, L, D, "k")
    WkT = [setup.tile([128, D], FP32, tag=f"WkT{i}", name=f"WkT{i}") for i in range(CB)]
    transpose_to(WkT, Wk_n, D, D, "wk")

    # ---------------- projections ----------------
    # All k-indexed tensors below use the permuted order k = 3r + j
    # (r = partition, j = sub-tile), matching the pos DMA layout where
    # partition r holds the 3 consecutive key rows [3r, 3r+3).  Softmax
    # and the output contraction are permutation-invariant in k as long
    # as kpT / e / v_aug / mask agree, which they do by construction.
    # q_projT [d', q] f32, then qu = +u, qv = +v (per-partition adds)
    quT, qvT = [], []
    for dt in range(2):
        ps = psum_big.tile([128, 512], FP32, tag="big", name="ps_projq")
        for cb in range(CB):
            nc.tensor.matmul(
                ps[:, :Q], WqT[cb][:, dt * 128:(dt + 1) * 128], qryT[cb],
                start=(cb == 0), stop=(cb == CB - 1))
        qp = setup.tile([128, Q], FP32, tag=f"qp{dt}", name=f"qp{dt}")
        nc.vector.tensor_scalar_add(out=qp, in0=ps[:, :Q], scalar1=bq_c[dt])
        qu = const.tile([128, Q], FP32, tag=f"qu{dt}", name=f"qu{dt}")
        nc.vector.tensor_scalar_add(out=qu, in0=qp, scalar1=u_c[dt])
        qv = const.tile([128, Q], FP32, tag=f"qv{dt}", name=f"qv{dt}")
        nc.vector.tensor_scalar_add(out=qv, in0=qp, scalar1=v_c[dt])
        quT.append(qu)
        qvT.append(qv)

    # per-head qv at partition base 0
    qv_h = [setup.tile([DH, Q], FP32, tag=f"qvh{h}", name=f"qvh{h}")
            for h in range(H)]
    for h in range(H):
        dt, r = h // 4, (h % 4) * DH
        nc.vector.tensor_copy(out=qv_h[h], in_=qvT[dt][r:r + DH, :])

    # ---------------- T32: B_D stationary weights ----------------
    # T32[cb] is [128 d, 96 q, 32 c] bf16: for query q, cols [32q, 32q+32)
    # hold T_q[d, h] at local col 8*(q%4)+h and zero elsewhere, so the
    # matmul T32_q^T @ posT_q lands on score partitions 8*(q%4)+h of the
    # query's 32-partition group.
    T32 = [const.tile([128, Q * 32], BF16, tag=f"T32_{cb}", name=f"T32_{cb}")
           for cb in range(CB)]
    for cb in range(CB):
        nc.vector.memset(T32[cb], 0.0)

    def emit_T32():
        for cb in range(CB):
            t32v = T32[cb].rearrange("p (t x) -> p t x", x=128)
            for h in range(H):
                # share the posT psum slots so the 16 T matmuls pipeline
                # instead of chasing 2 "big" slots
                ps = psum_posT.tile([128, 512], FP32, tag="pt", name="ps_T")
                nc.tensor.matmul(
                    ps[:, :Q], Wr_h[h][:, cb * 128:(cb + 1) * 128],
                    qv_h[h], start=True, stop=True)
                # dst cols 128t + 40j + h over (t, j): stride-40 step slice
                if h % 2 == 0:
                    nc.vector.tensor_copy(
                        out=t32v[:, :, h::40],
                        in_=ps[:, :Q].rearrange("p (t j) -> p t j", j=4))
                else:
                    nc.scalar.activation(
                        out=t32v[:, :, h::40],
                        in_=ps[:, :Q].rearrange("p (t j) -> p t j", j=4),
                        func=mybir.ActivationFunctionType.Copy)

    # ---------------- qu_bd: batched A_C stationary weights ----------
    # qu_bd[cb] is [128 d', 6 g, 128 (4J,4j,8h)] bf16: col (g,J,j,h) holds
    # (q+u)[d', q=16g+4J+j] on head h's 32-row diagonal block, 0 elsewhere.
    qu_bd = [const.tile([128, NG * 128], BF16, tag=f"qbd{cb}",
                        name=f"qbd{cb}") for cb in range(CB)]
    for cb in range(CB):
        nc.vector.memset(qu_bd[cb], 0.0)
        qbv = qu_bd[cb].rearrange(
            "p (g J j x) -> p g J j x", J=4, j=4, x=H)
        for hl in range(4):
            h = cb * 4 + hl
            nc.vector.tensor_copy(
                out=qbv[hl * DH:(hl + 1) * DH, :, :, :, h],
                in_=quT[cb][hl * DH:(hl + 1) * DH, :].rearrange(
                    "p (g J j) -> p g J j", J=4, j=4))

    # k_projT [d', k] bf16, full [128, L] tiles (contraction layout for
    # A_C) -- emitted inside the main loop after group 0's transposes, so
    # the PE starts on pos data as soon as the first slab lands.
    kpT = [setup.tile([128, L], BF16, tag=f"kpT{i}", name=f"kpT{i}")
           for i in range(CB)]

    def emit_kpT():
        for dt in range(2):
            ps = psum_big.tile([128, 512], FP32, tag="big", name="ps_proj")
            for cb in range(CB):
                nc.tensor.matmul(
                    ps[:, :L], WkT[cb][:, dt * 128:(dt + 1) * 128],
                    keyT[cb].rearrange("p (r j) -> p j r", j=KT),
                    start=(cb == 0), stop=(cb == CB - 1))
            nc.vector.tensor_scalar_add(
                out=kpT[dt], in0=ps[:, :L], scalar1=bk_c[dt])

    # v_proj natural [k, d'] + ones column per head -> v_aug [128, H, DH+1]
    # bf16; rows scaled by key mask (folds both the -1e15 mask bias and the
    # softmax denominator's mask into the output matmul).  Only the output
    # stage needs it, so it is emitted mid-loop after group 0.
    ones_1 = const.tile([1, D], FP32)
    nc.vector.memset(ones_1, 1.0)
    valT = [setup.tile([128, L], FP32, tag=f"valT{i}", name=f"valT{i}")
            for i in range(CB)]
    WvT = [setup.tile([128, D], FP32, tag=f"WvT{i}", name=f"WvT{i}")
           for i in range(CB)]
    v_aug = [const.tile([128, H, DH + 1], BF16, tag=f"va{j}", name=f"va{j}")
             for j in range(KT)]

    def emit_vpath():
        transpose_to(valT, val_n, L, D, "v")
        transpose_to(WvT, Wv_n, D, D, "wv")
        for j in range(KT):
            ps = psum_big.tile([128, 512], FP32, tag="big", name="ps_projv")
            for cb in range(CB):
                nc.tensor.matmul(
                    ps[:, :D],
                    valT[cb].rearrange("p (r j) -> p j r", j=KT)[:, j],
                    WvT[cb],
                    start=(cb == 0), stop=False)
            # + bias bv broadcast over rows (rank-1 matmul with ones lhsT)
            nc.tensor.matmul(ps[:, :D], ones_1[:, :128], bv_row,
                             start=False, stop=True)
            va = v_aug[j]
            nc.vector.memset(va, 1.0)
            nc.vector.tensor_copy(
                out=va[:, :, 0:DH],
                in_=ps[:, :D].rearrange("p (h d) -> p h d", h=H))
            nc.vector.tensor_scalar_mul(
                out=va, in0=va, scalar1=mask_p[:, j:j + 1])

    # ---------------- eT: exp(scores) transposed, [k, (h,q)] ----------
    eT = [const.tile([128, H * Q], BF16, tag=f"eT{kt}", name=f"eT{kt}")
          for kt in range(KT)]

    # ---------------- main loop: score groups of 16 queries ------------
    # Per group: stream 16 pairs of transposes + PSUM->SBUF copies first
    # (PE never waits on DVE/ACT), then run the batched A_C + 32 B_D
    # matmuls over the buffered pT tiles, then exp + e-transpose.
    slab = [None]

    def emit_transposes(q):
        """Transpose pair q's pos slab; returns the 2 buffered pT tiles."""
        i = q % PG
        pTs = []
        for cb in range(CB):
            ps = psum_posT.tile([128, 1024], BF16, tag="pt", name="pt_ps")
            for j in range(KT):
                nc.tensor.matmul(
                    ps[:, j * 128:(j + 1) * 128],
                    slab[0][:, i, j, cb * 128:(cb + 1) * 128],
                    ident_b,
                    is_transpose=True,
                    start=(j == 0), stop=(j == KT - 1))
            pT = pt_pool.tile([128, L], BF16, tag=f"posT{cb}",
                              name=f"posT{cb}", bufs=22)
            if cb == 0:
                nc.vector.tensor_copy(out=pT, in_=ps[:, :L])
            else:
                nc.scalar.activation(
                    out=pT, in_=ps[:, :L],
                    func=mybir.ActivationFunctionType.Copy)
            pTs.append(pT)
        return pTs

    def emit_eT(g, e):
        """e-transpose for a finished score group (deferred one group so
        PE never waits on the exp)."""
        for kt in range(KT):
            pe = psum_big.tile([128, 1024], BF16, tag="eTp", name="pe",
                               bufs=1)
            nc.tensor.matmul(
                pe[:, :128], e[:, kt * 128:(kt + 1) * 128], ident_b,
                is_transpose=True, start=True, stop=True)
            dst = eT[kt].rearrange(
                "p (h g J j) -> p g J j h", h=H, g=NG, J=4, j=4)[:, g]
            nc.vector.tensor_copy(
                out=dst,
                in_=pe[:, :128].rearrange("p (J j h) -> p J j h", J=4, j=4))

    pending_eT = None
    for g in range(NG):
        pT_buf = []
        for jj in range(16):
            q = g * 16 + jj
            if q % PG == 0:
                # SWDGE cast-DMA: fp32 HBM -> bf16 SBUF, 3KB-contiguous
                # descriptors (partition r holds key rows [3r, 3r+3)).
                s = pair_pool.tile([128, PG, KT, D], BF16, tag="slab",
                                   name="slab", bufs=10)
                if q < 10 * PG:
                    # WAW-gate the pre-loop slabs behind the last critical
                    # sync-ring DMA, so these big transfers don't starve
                    # the small setup loads on the shared SDMA engines
                    # (the scheduler would otherwise hoist them).
                    nc.gpsimd.tensor_copy(
                        out=s[0:1, 0, 0, 4:8], in_=key_n[2][0:1, 0:4])
                nc.gpsimd.dma_start(
                    out=s,
                    in_=pos[q:q + PG].rearrange(
                        "g (r j) c -> r g j c", j=KT))
                slab[0] = s
            pT_buf.append(emit_transposes(q))
        if g == 0:
            emit_T32()
            emit_kpT()
        elif g == 1:
            emit_vpath()
        if pending_eT is not None:
            emit_eT(*pending_eT)
        sc = psum_sc.tile([128, 512], FP32, tag="sc", name=f"sc{g}")
        scv = sc[:, :L]
        for cb in range(CB):
            nc.tensor.matmul(
                scv, qu_bd[cb][:, g * 128:(g + 1) * 128], kpT[cb],
                start=(cb == 0), stop=(cb == CB - 1))
        for jj in range(16):
            q = g * 16 + jj
            J = jj // 4
            for cb in range(CB):
                # start/stop bookkeeping lives on the A_C matmuls (which
                # cover all 128 partitions); on HW stop is a no-op and
                # accumulation is per-element, so skip the group check.
                nc.tensor.matmul(
                    scv[J * 32:(J + 1) * 32, :],
                    T32[cb][:, q * 32:(q + 1) * 32],
                    pT_buf[jj][cb],
                    start=False, stop=False, skip_group_check=True,
                    tile_position=(0, J * 32))
        e = pair_pool.tile([128, L], BF16, tag="e", name=f"e{g}")
        nc.scalar.activation(
            out=e, in_=scv, func=mybir.ActivationFunctionType.Exp,
            scale=float(SCALE))
        pending_eT = (g, e)
    emit_eT(*pending_eT)

    # ---------------- output matmuls + normalize ----------------
    # Batched phases with 4 heads packed per PSUM bank, so the 8 heads
    # don't serialize through a single po -> copy -> transpose -> recip
    # latency chain.
    out_sb = setup.tile([96, D], FP32, tag="osb")
    tmp4 = [pt_pool.tile([DH + 1, 4 * Q], FP32, tag=f"otmp{t}",
                         name=f"otmp{t}") for t in range(2)]
    pot4 = []
    for t in range(2):
        po = psum_big.tile([DH + 1, 512], FP32, tag="big", name=f"po{t}")
        for hh in range(4):
            h = t * 4 + hh
            for kt in range(KT):
                nc.tensor.matmul(
                    po[:, hh * Q:(hh + 1) * Q],
                    v_aug[kt][:, h, :], eT[kt][:, h * Q:(h + 1) * Q],
                    start=(hh == 0 and kt == 0),
                    stop=(hh == 3 and kt == KT - 1),
                    skip_group_check=True)
        nc.vector.tensor_copy(out=tmp4[t], in_=po[:, :4 * Q])
        pot = psum_posT.tile([Q, 512], FP32, tag="pt", name=f"pot{t}")
        for hh in range(4):
            nc.tensor.matmul(
                pot[:, hh * (DH + 1):(hh + 1) * (DH + 1)],
                tmp4[t][:, hh * Q:(hh + 1) * Q],
                ident_f[:DH + 1, :DH + 1],
                is_transpose=True,
                start=(hh == 0), stop=(hh == 3), skip_group_check=True)
        pot4.append(pot)
    for t in range(2):
        rec = pt_pool.tile([Q, 4], FP32, tag="rec", name=f"rec{t}")
        nc.vector.reciprocal(
            out=rec,
            in_=pot4[t][:, :4 * (DH + 1)].rearrange(
                "p (hh x) -> p hh x", x=DH + 1)[:, :, DH])
        for hh in range(4):
            h = t * 4 + hh
            nc.vector.tensor_scalar_mul(
                out=out_sb[:, h * DH:(h + 1) * DH],
                in0=pot4[t][:, hh * (DH + 1):hh * (DH + 1) + DH],
                scalar1=rec[:, hh:hh + 1])

    nc.sync.dma_start(out=out, in_=out_sb)
    ctx.close()


def build_program():
    nc = bacc.Bacc(
        "TRN2", target_bir_lowering=False, debug=False,
        num_devices=NCORES)
    ins = {
        "pos": nc.dram_tensor("pos", [Q, L, D], FP32, kind="ExternalInput").ap(),
        "key": nc.dram_tensor("key", [L, D], FP32, kind="ExternalInput").ap(),
        "value": nc.dram_tensor("value", [L, D], FP32, kind="ExternalInput").ap(),
        "query": nc.dram_tensor("query", [Q, D], FP32, kind="ExternalInput").ap(),
        "mask": nc.dram_tensor("mask", [L], FP32, kind="ExternalInput").ap(),
        "Wk": nc.dram_tensor("Wk", [D, D], FP32, kind="ExternalInput").ap(),
        "Wq": nc.dram_tensor("Wq", [D, D], FP32, kind="ExternalInput").ap(),
        "Wv": nc.dram_tensor("Wv", [D, D], FP32, kind="ExternalInput").ap(),
        "Wr": nc.dram_tensor("Wr", [D, D], FP32, kind="ExternalInput").ap(),
        "bk": nc.dram_tensor("bk", [D], FP32, kind="ExternalInput").ap(),
        "bq": nc.dram_tensor("bq", [D], FP32, kind="ExternalInput").ap(),
        "bv": nc.dram_tensor("bv", [D], FP32, kind="ExternalInput").ap(),
        "u": nc.dram_tensor("u", [H, DH], FP32, kind="ExternalInput").ap(),
        "v": nc.dram_tensor("v", [H, DH], FP32, kind="ExternalInput").ap(),
    }
    outs = {
        "out": nc.dram_tensor("out", [Q, D], FP32, kind="ExternalOutput").ap(),
    }
    with tile.TileContext(nc) as tc:
        build_kernel_body(tc, outs, ins)
    nc.compile()
    return nc


def shard_inputs(inputs):
    """Full inputs -> list of 8 per-core input dicts (numpy, contiguous)."""
    f32 = lambda a: np.ascontiguousarray(np.asarray(a), dtype=np.float32)
    pos = f32(inputs["pos"])
    key = f32(inputs["key"])
    query = f32(inputs["query"])
    value = f32(inputs["value"])
    mask = f32(inputs["key_mask"])
    shared = {
        "Wk": f32(inputs["Wk"]), "Wq": f32(inputs["Wq"]),
        "Wv": f32(inputs["Wv"]), "Wr": f32(inputs["Wr"]),
        "bk": f32(inputs["bk"]), "bq": f32(inputs["bq"]),
        "bv": f32(inputs["bv"]),
        "u": f32(inputs["u"]), "v": f32(inputs["v"]),
    }
    in_maps = []
    for c in range(NCORES):
        b, q0 = c // 4, (c % 4) * Q
        m = dict(shared)
        m["pos"] = np.ascontiguousarray(pos[b, q0:q0 + Q])
        m["key"] = key[b]
        m["value"] = value[b]
        m["query"] = np.ascontiguousarray(query[b, q0:q0 + Q])
        m["mask"] = mask[b]
        in_maps.append(m)
    return in_maps


_CACHED = {}


def kernel(**inputs):
    from concourse.bass_utils import run_bass_kernel_spmd

    if "nc" not in _CACHED:
        _CACHED["nc"] = build_program()
    nc = _CACHED["nc"]
    in_maps = shard_inputs(inputs)
    res = run_bass_kernel_spmd(nc, in_maps, core_ids=list(range(NCORES)))
    out = np.zeros((B, L, D), dtype=np.float32)
    for c in range(NCORES):
        b, q0 = c // 4, (c % 4) * Q
        out[b, q0:q0 + Q] = res.results[c]["out"]
    return out


# revision 54
# speedup vs baseline: 1.0144x; 1.0144x over previous
"""Trainium2 Bass kernel for relative-position multi-head attention.

Shapes (hardcoded): B=2, L=384, D=256, H=8, DH=32.
Sharding: 8 cores; core c handles batch b=c//4, query rows [(c%4)*96, +96).
Pure data-parallel SPMD - no collectives.

Math (per batch b, query q):
  q/k/v projections: x @ W.T + bias
  A_C[h,k] = (q_h+u_h) . k_h[k]
  B_D[h,k] = (q_h+v_h) . (Wr_h @ pos[q,k] + br_h)
           = (Wr_h^T (q_h+v_h)) . pos[q,k]   + const(h,q)   [br term is
             k-independent -> cancels in softmax -> dropped]
  score    = (A_C + B_D)/sqrt(DH) - (1-mask[k])*1e15
  out      = softmax_k(score) @ v

Key restructurings for the hardware (v2):
  * r = pos @ Wr.T (38 GFLOP) is never materialized; instead
    T[q] = Wr^T-blockdiag @ (q+v)  (a [256,8] matrix per query) and
    B_D = T^T @ posT  (1.2 GFLOP).
  * pos stays fp32 end-to-end on the DMA/cast path: the PE transposes it
    chunk-wise (float32r transpose mode), and the PSUM->SBUF copy does
    the fp32->bf16 cast on DVE/ACT.  The gpsimd engine (the v1
    bottleneck: 250us of fp32->bf16 casts) does almost nothing.
  * scores live in PSUM as [(16q x 8h) partitions, 384 k free]; per
    16-query group one psum tile gets: 2 batched A_C matmuls (block-
    diagonal (q+u) weights vs k_projT, moving N=384) + per-query B_D
    matmuls (T stationary 32 cols, posT moving N=384).
  * softmax over k (free dim): one exp per group on ACT; the key mask
    and the softmax denominator are folded into an augmented, mask-
    scaled value matrix so normalization falls out of the output matmul.
"""

import sys

for _p in ("/opt/trn_rl_repo", "/root/.axon_site/_ro/trn_rl_repo"):
    if _p not in sys.path:
        sys.path.append(_p)

import numpy as np

import concourse.bass as bass
import concourse.mybir as mybir
import concourse.tile as tile
from concourse import bacc
from concourse.masks import make_identity

FP32 = mybir.dt.float32
FP32R = mybir.dt.float32r
BF16 = mybir.dt.bfloat16

B, L, D, H = 2, 384, 256, 8
DH = D // H            # 32
Q = 96                 # queries per core
KT = L // 128          # 3 k-tiles
CB = D // 128          # 2 contraction blocks
NCORES = 8
PG = 4                 # pairs per DMA batch
NG = Q // 16           # score groups of 16 queries
SCALE = 1.0 / np.sqrt(DH)


def build_kernel_body(tc, outs, ins):
    """Emit the per-core program. outs/ins are dicts of DRAM APs."""
    from contextlib import ExitStack
    ctx = ExitStack()
    pool = lambda **kw: ctx.enter_context(tc.tile_pool(**kw))
    nc = tc.nc
    pos = ins["pos"]          # [Q, L, D] f32
    key = ins["key"]          # [L, D]
    value = ins["value"]      # [L, D]
    query = ins["query"]      # [Q, D]
    mask = ins["mask"]        # [L]
    Wk, Wq, Wv, Wr = ins["Wk"], ins["Wq"], ins["Wv"], ins["Wr"]   # [D, D]
    bk, bq, bv = ins["bk"], ins["bq"], ins["bv"]                  # [D]
    u_in, v_in = ins["u"], ins["v"]                               # [H, DH]
    out = outs["out"]         # [Q, D] f32

    const = pool(name="const", bufs=1)
    setup = pool(name="setup", bufs=2)
    psum_big = pool(name="psum_big", bufs=2, space="PSUM")
    psum_sc = pool(name="psum_sc", bufs=2, space="PSUM")
    psum_posT = pool(name="psum_posT", bufs=3, space="PSUM")
    pair_pool = pool(name="pair", bufs=3)
    pt_pool = pool(name="pt", bufs=4)

    # ---------------- identities ----------------
    ident_f = const.tile([128, 128], FP32)
    make_identity(nc, ident_f)
    ident_b = const.tile([128, 128], BF16)
    nc.vector.tensor_copy(out=ident_b, in_=ident_f)

    # ---------------- load weights + inputs ----------------
    # One consolidated DMA per tensor; T32's dependency chain (Wq -> q-proj
    # -> qv -> T) is loaded first.  Small column loads go on the ACT HWDGE
    # queue so the Pool queue is free to start emitting pos slab DMAs.
    def load_fold(ap, rows, tg, eng=nc.sync):  # [rows, D] dram -> [128, rows//128, D]
        n = rows // 128
        t = setup.tile([128, n, D], FP32, tag=f"ld_{tg}", name=f"ld_{tg}")
        eng.dma_start(out=t, in_=ap.rearrange("(i p) c -> p i c", p=128))
        return [t[:, i, :] for i in range(n)]

    key_n = load_fold(key, L, "key")
    qry_n = setup.tile([96, D], FP32)
    nc.sync.dma_start(out=qry_n, in_=query)
    Wq_n = load_fold(Wq, D, "wq")
    # Wr as [32 dh, 8 h, 256]: per-head lhsT slices at partition base 0
    wr_t = const.tile([DH, H, D], FP32)
    nc.sync.dma_start(
        out=wr_t, in_=Wr.rearrange("(h dh) c -> dh h c", dh=DH))
    Wr_h = [wr_t[:, h, :] for h in range(H)]
    # (slab DMA gate is emitted below, after the last setup DMA)

    def col_load(ap1d, n, tag):  # [n] dram -> list of [128,1] sbuf columns
        t = const.tile([128, n // 128], FP32, tag=f"col_{tag}",
                       name=f"col_{tag}")
        nc.scalar.dma_start(
            out=t, in_=ap1d.rearrange("(i p) -> p i", p=128))
        return [t[:, i:i + 1] for i in range(n // 128)]

    bq_c = col_load(bq, D, "bq")
    u_c = col_load(u_in.rearrange("h d -> (h d)"), D, "u")
    v_c = col_load(v_in.rearrange("h d -> (h d)"), D, "v")
    bk_c = col_load(bk, D, "bk")
    bv_row = const.tile([1, D], FP32)
    nc.scalar.dma_start(out=bv_row, in_=bv.rearrange("(o d) -> o d", o=1))
    # mask columns in permuted order: mask_p[r, j] = mask[3r + j]
    mask_p = const.tile([128, KT], FP32)
    nc.scalar.dma_start(
        out=mask_p, in_=mask.rearrange("(r j) -> r j", j=KT))
    # value path loads last: v_aug is only needed by the output stage
    Wk_n = load_fold(Wk, D, "wk", eng=nc.scalar)
    val_n = load_fold(value, L, "val", eng=nc.scalar)
    Wv_n = load_fold(Wv, D, "wv", eng=nc.scalar)


    # ---------------- transpose helper (fp32, PE) ----------------
    def transpose_to(dst_tiles, src_tiles, rows, cols, tag):
        """src: list of sbuf tiles [<=128, cols] covering [rows, cols].
        dst_tiles: list of CB sbuf tiles [128, rows] covering [cols, rows]."""
        for cb in range(cols // 128):
            ps = psum_big.tile([128, 512], FP32, tag="big", name="ps_tp")
            nrt = len(src_tiles)
            for i, st in enumerate(src_tiles):
                r = st.shape[0]
                nc.tensor.matmul(
                    ps[:, i * 128:i * 128 + r],
                    st[:, cb * 128:(cb + 1) * 128],
                    ident_f[:r, :r],
                    is_transpose=True,
                    start=(i == 0), stop=(i == nrt - 1))
            nc.vector.tensor_copy(out=dst_tiles[cb], in_=ps[:, :rows])

    qryT = [setup.tile([128, Q], FP32, tag=f"qryT{i}", name=f"qryT{i}") for i in range(CB)]
    transpose_to(qryT, [qry_n], Q, D, "q")
    WqT = [setup.tile([128, D], FP32, tag=f"WqT{i}", name=f"WqT{i}") for i in range(CB)]
    transpose_to(WqT, Wq_n, D, D, "wq")
    keyT = [setup.tile([128, L], FP32, tag=f"keyT{i}", name=f"keyT{i}") for i in range(CB)]
    transpose_to(keyT, key_n, L, D, "k")
    WkT = [setup.tile([128, D], FP32, tag=f"WkT{i}", name=f"WkT{i}") for i in range(CB)]
    transpose_to(WkT, Wk_n, D, D, "wk")

    # ---------------- projections ----------------
    # All k-indexed tensors below use the permuted order k = 3r + j
    # (r = partition, j = sub-tile), matching the pos DMA layout where
    # partition r holds the 3 consecutive key rows [3r, 3r+3).  Softmax
    # and the output contraction are permutation-invariant in k as long
    # as kpT / e / v_aug / mask agree, which they do by construction.
    # q_projT [d', q] f32, then qu = +u, qv = +v (per-partition adds)
    quT, qvT = [], []
    for dt in range(2):
        ps = psum_big.tile([128, 512], FP32, tag="big", name="ps_projq")
        for cb in range(CB):
            nc.tensor.matmul(
                ps[:, :Q], WqT[cb][:, dt * 128:(dt + 1) * 128], qryT[cb],
                start=(cb == 0), stop=(cb == CB - 1))
        qp = setup.tile([128, Q], FP32, tag=f"qp{dt}", name=f"qp{dt}")
        nc.vector.tensor_scalar_add(out=qp, in0=ps[:, :Q], scalar1=bq_c[dt])
        qu = const.tile([128, Q], FP32, tag=f"qu{dt}", name=f"qu{dt}")
        nc.vector.tensor_scalar_add(out=qu, in0=qp, scalar1=u_c[dt])
        qv = const.tile([128, Q], FP32, tag=f"qv{dt}", name=f"qv{dt}")
        nc.vector.tensor_scalar_add(out=qv, in0=qp, scalar1=v_c[dt])
        quT.append(qu)
        qvT.append(qv)

    # per-head qv at partition base 0
    qv_h = [setup.tile([DH, Q], FP32, tag=f"qvh{h}", name=f"qvh{h}")
            for h in range(H)]
    for h in range(H):
        dt, r = h // 4, (h % 4) * DH
        nc.vector.tensor_copy(out=qv_h[h], in_=qvT[dt][r:r + DH, :])

    # ---------------- T32: B_D stationary weights ----------------
    # T32[cb] is [128 d, 96 q, 32 c] bf16: for query q, cols [32q, 32q+32)
    # hold T_q[d, h] at local col 8*(q%4)+h and zero elsewhere, so the
    # matmul T32_q^T @ posT_q lands on score partitions 8*(q%4)+h of the
    # query's 32-partition group.
    T32 = [const.tile([128, Q * 32], BF16, tag=f"T32_{cb}", name=f"T32_{cb}")
           for cb in range(CB)]
    for cb in range(CB):
        nc.vector.memset(T32[cb], 0.0)

    def emit_T32():
        # emitted in setup: moving it into the loop contends with the pair
        # transposes for the pt psum slots and loses ~3us
        for cb in range(CB):
            t32v = T32[cb].rearrange("p (t x) -> p t x", x=128)
            for h in range(H):
                # share the posT psum slots so the 16 T matmuls pipeline
                # instead of chasing 2 "big" slots
                ps = psum_posT.tile([128, 512], FP32, tag="pt", name="ps_T")
                nc.tensor.matmul(
                    ps[:, :Q], Wr_h[h][:, cb * 128:(cb + 1) * 128],
                    qv_h[h], start=True, stop=True)
                # dst cols 128t + 40j + h over (t, j): stride-40 step slice
                if h % 2 == 0:
                    nc.vector.tensor_copy(
                        out=t32v[:, :, h::40],
                        in_=ps[:, :Q].rearrange("p (t j) -> p t j", j=4))
                else:
                    nc.scalar.activation(
                        out=t32v[:, :, h::40],
                        in_=ps[:, :Q].rearrange("p (t j) -> p t j", j=4),
                        func=mybir.ActivationFunctionType.Copy)

    emit_T32()

    # ---------------- qu_bd: batched A_C stationary weights ----------
    # qu_bd[cb] is [128 d', 6 g, 128 (4J,4j,8h)] bf16: col (g,J,j,h) holds
    # (q+u)[d', q=16g+4J+j] on head h's 32-row diagonal block, 0 elsewhere.
    qu_bd = [const.tile([128, NG * 128], BF16, tag=f"qbd{cb}",
                        name=f"qbd{cb}") for cb in range(CB)]
    for cb in range(CB):
        nc.vector.memset(qu_bd[cb], 0.0)
        qbv = qu_bd[cb].rearrange(
            "p (g J j x) -> p g J j x", J=4, j=4, x=H)
        for hl in range(4):
            h = cb * 4 + hl
            nc.vector.tensor_copy(
                out=qbv[hl * DH:(hl + 1) * DH, :, :, :, h],
                in_=quT[cb][hl * DH:(hl + 1) * DH, :].rearrange(
                    "p (g J j) -> p g J j", J=4, j=4))

    # k_projT [d', k] bf16, full [128, L] tiles (contraction layout for
    # A_C) -- emitted inside the main loop after group 0's transposes, so
    # the PE starts on pos data as soon as the first slab lands.
    kpT = [setup.tile([128, L], BF16, tag=f"kpT{i}", name=f"kpT{i}")
           for i in range(CB)]

    def emit_kpT():
        for dt in range(2):
            ps = psum_big.tile([128, 512], FP32, tag="big", name="ps_proj")
            for cb in range(CB):
                nc.tensor.matmul(
                    ps[:, :L], WkT[cb][:, dt * 128:(dt + 1) * 128],
                    keyT[cb].rearrange("p (r j) -> p j r", j=KT),
                    start=(cb == 0), stop=(cb == CB - 1))
            nc.vector.tensor_scalar_add(
                out=kpT[dt], in0=ps[:, :L], scalar1=bk_c[dt])

    # v_proj natural [k, d'] + ones column per head -> v_aug [128, H, DH+1]
    # bf16; rows scaled by key mask (folds both the -1e15 mask bias and the
    # softmax denominator's mask into the output matmul).  Only the output
    # stage needs it, so it is emitted mid-loop after group 0.
    ones_1 = const.tile([1, D], FP32)
    nc.vector.memset(ones_1, 1.0)
    valT = [setup.tile([128, L], FP32, tag=f"valT{i}", name=f"valT{i}")
            for i in range(CB)]
    WvT = [setup.tile([128, D], FP32, tag=f"WvT{i}", name=f"WvT{i}")
           for i in range(CB)]
    v_aug = [const.tile([128, H, DH + 1], BF16, tag=f"va{j}", name=f"va{j}")
             for j in range(KT)]

    def emit_vpath():
        transpose_to(valT, val_n, L, D, "v")
        transpose_to(WvT, Wv_n, D, D, "wv")
        for j in range(KT):
            ps = psum_big.tile([128, 512], FP32, tag="big", name="ps_projv")
            for cb in range(CB):
                nc.tensor.matmul(
                    ps[:, :D],
                    valT[cb].rearrange("p (r j) -> p j r", j=KT)[:, j],
                    WvT[cb],
                    start=(cb == 0), stop=False)
            # + bias bv broadcast over rows (rank-1 matmul with ones lhsT)
            nc.tensor.matmul(ps[:, :D], ones_1[:, :128], bv_row,
                             start=False, stop=True)
            va = v_aug[j]
            nc.vector.memset(va, 1.0)
            nc.vector.tensor_copy(
                out=va[:, :, 0:DH],
                in_=ps[:, :D].rearrange("p (h d) -> p h d", h=H))
            nc.vector.tensor_scalar_mul(
                out=va, in0=va, scalar1=mask_p[:, j:j + 1])

    # ---------------- eT: exp(scores) transposed, [k, (h,q)] ----------
    eT = [const.tile([128, H * Q], BF16, tag=f"eT{kt}", name=f"eT{kt}")
          for kt in range(KT)]

    # ---------------- main loop: score groups of 16 queries ------------
    # Per group: stream 16 pairs of transposes + PSUM->SBUF copies first
    # (PE never waits on DVE/ACT), then run the batched A_C + 32 B_D
    # matmuls over the buffered pT tiles, then exp + e-transpose.
    slab = [None]

    def emit_transposes(q):
        """Transpose pair q's pos slab; returns the 2 buffered pT tiles."""
        i = q % PG
        pTs = []
        for cb in range(CB):
            ps = psum_posT.tile([128, 1024], BF16, tag="pt", name="pt_ps")
            for j in range(KT):
                nc.tensor.matmul(
                    ps[:, j * 128:(j + 1) * 128],
                    slab[0][:, i, j, cb * 128:(cb + 1) * 128],
                    ident_b,
                    is_transpose=True,
                    start=(j == 0), stop=(j == KT - 1))
            pT = pt_pool.tile([128, L], BF16, tag=f"posT{cb}",
                              name=f"posT{cb}", bufs=22)
            if cb == 0:
                nc.vector.tensor_copy(out=pT, in_=ps[:, :L])
            else:
                nc.scalar.activation(
                    out=pT, in_=ps[:, :L],
                    func=mybir.ActivationFunctionType.Copy)
            pTs.append(pT)
        return pTs

    def emit_eT(g, e):
        """e-transpose for a finished score group (deferred one group so
        PE never waits on the exp)."""
        for kt in range(KT):
            pe = psum_big.tile([128, 1024], BF16, tag="eTp", name="pe",
                               bufs=1)
            nc.tensor.matmul(
                pe[:, :128], e[:, kt * 128:(kt + 1) * 128], ident_b,
                is_transpose=True, start=True, stop=True)
            dst = eT[kt].rearrange(
                "p (h g J j) -> p g J j h", h=H, g=NG, J=4, j=4)[:, g]
            nc.vector.tensor_copy(
                out=dst,
                in_=pe[:, :128].rearrange("p (J j h) -> p J j h", J=4, j=4))

    pending_eT = None
    for g in range(NG):
        pT_buf = []
        for jj in range(16):
            q = g * 16 + jj
            if q % PG == 0:
                # SWDGE cast-DMA: fp32 HBM -> bf16 SBUF, 3KB-contiguous
                # descriptors (partition r holds key rows [3r, 3r+3)).
                s = pair_pool.tile([128, PG, KT, D], BF16, tag="slab",
                                   name="slab", bufs=10)
                if q < 10 * PG:
                    # WAW-gate the pre-loop slabs behind the last critical
                    # sync-ring DMA, so these big transfers don't starve
                    # the small setup loads on the shared SDMA engines
                    # (the scheduler would otherwise hoist them).
                    nc.gpsimd.tensor_copy(
                        out=s[0:1, 0, 0, 4:8], in_=key_n[2][0:1, 0:4])
                nc.gpsimd.dma_start(
                    out=s,
                    in_=pos[q:q + PG].rearrange(
                        "g (r j) c -> r g j c", j=KT))
                slab[0] = s
            pT_buf.append(emit_transposes(q))
        if g == 0:
            emit_kpT()
        elif g == 1:
            emit_vpath()
        if pending_eT is not None:
            emit_eT(*pending_eT)
        sc = psum_sc.tile([128, 512], FP32, tag="sc", name=f"sc{g}")
        scv = sc[:, :L]
        for cb in range(CB):
            nc.tensor.matmul(
                scv, qu_bd[cb][:, g * 128:(g + 1) * 128], kpT[cb],
                start=(cb == 0), stop=(cb == CB - 1))
        for jj in range(16):
            q = g * 16 + jj
            J = jj // 4
            for cb in range(CB):
                # start/stop bookkeeping lives on the A_C matmuls (which
                # cover all 128 partitions); on HW stop is a no-op and
                # accumulation is per-element, so skip the group check.
                nc.tensor.matmul(
                    scv[J * 32:(J + 1) * 32, :],
                    T32[cb][:, q * 32:(q + 1) * 32],
                    pT_buf[jj][cb],
                    start=False, stop=False, skip_group_check=True,
                    tile_position=(0, J * 32))
        e = pair_pool.tile([128, L], BF16, tag="e", name=f"e{g}")
        nc.scalar.activation(
            out=e, in_=scv, func=mybir.ActivationFunctionType.Exp,
            scale=float(SCALE))
        pending_eT = (g, e)
    emit_eT(*pending_eT)

    # ---------------- output matmuls + normalize ----------------
    # Batched phases with 4 heads packed per PSUM bank, so the 8 heads
    # don't serialize through a single po -> copy -> transpose -> recip
    # latency chain.
    out_sb = setup.tile([96, D], FP32, tag="osb")
    tmp4 = [pt_pool.tile([DH + 1, 4 * Q], FP32, tag=f"otmp{t}",
                         name=f"otmp{t}") for t in range(2)]
    pot4 = []
    for t in range(2):
        po = psum_big.tile([DH + 1, 512], FP32, tag="big", name=f"po{t}")
        for hh in range(4):
            h = t * 4 + hh
            for kt in range(KT):
                nc.tensor.matmul(
                    po[:, hh * Q:(hh + 1) * Q],
                    v_aug[kt][:, h, :], eT[kt][:, h * Q:(h + 1) * Q],
                    start=(hh == 0 and kt == 0),
                    stop=(hh == 3 and kt == KT - 1),
                    skip_group_check=True)
        nc.vector.tensor_copy(out=tmp4[t], in_=po[:, :4 * Q])
        pot = psum_posT.tile([Q, 512], FP32, tag="pt", name=f"pot{t}")
        for hh in range(4):
            nc.tensor.matmul(
                pot[:, hh * (DH + 1):(hh + 1) * (DH + 1)],
                tmp4[t][:, hh * Q:(hh + 1) * Q],
                ident_f[:DH + 1, :DH + 1],
                is_transpose=True,
                start=(hh == 0), stop=(hh == 3), skip_group_check=True)
        pot4.append(pot)
    for t in range(2):
        rec = pt_pool.tile([Q, 4], FP32, tag="rec", name=f"rec{t}")
        nc.vector.reciprocal(
            out=rec,
            in_=pot4[t][:, :4 * (DH + 1)].rearrange(
                "p (hh x) -> p hh x", x=DH + 1)[:, :, DH])
        for hh in range(4):
            h = t * 4 + hh
            nc.vector.tensor_scalar_mul(
                out=out_sb[:, h * DH:(h + 1) * DH],
                in0=pot4[t][:, hh * (DH + 1):hh * (DH + 1) + DH],
                scalar1=rec[:, hh:hh + 1])

    nc.sync.dma_start(out=out, in_=out_sb)
    ctx.close()


def build_program():
    nc = bacc.Bacc(
        "TRN2", target_bir_lowering=False, debug=False,
        num_devices=NCORES)
    ins = {
        "pos": nc.dram_tensor("pos", [Q, L, D], FP32, kind="ExternalInput").ap(),
        "key": nc.dram_tensor("key", [L, D], FP32, kind="ExternalInput").ap(),
        "value": nc.dram_tensor("value", [L, D], FP32, kind="ExternalInput").ap(),
        "query": nc.dram_tensor("query", [Q, D], FP32, kind="ExternalInput").ap(),
        "mask": nc.dram_tensor("mask", [L], FP32, kind="ExternalInput").ap(),
        "Wk": nc.dram_tensor("Wk", [D, D], FP32, kind="ExternalInput").ap(),
        "Wq": nc.dram_tensor("Wq", [D, D], FP32, kind="ExternalInput").ap(),
        "Wv": nc.dram_tensor("Wv", [D, D], FP32, kind="ExternalInput").ap(),
        "Wr": nc.dram_tensor("Wr", [D, D], FP32, kind="ExternalInput").ap(),
        "bk": nc.dram_tensor("bk", [D], FP32, kind="ExternalInput").ap(),
        "bq": nc.dram_tensor("bq", [D], FP32, kind="ExternalInput").ap(),
        "bv": nc.dram_tensor("bv", [D], FP32, kind="ExternalInput").ap(),
        "u": nc.dram_tensor("u", [H, DH], FP32, kind="ExternalInput").ap(),
        "v": nc.dram_tensor("v", [H, DH], FP32, kind="ExternalInput").ap(),
    }
    outs = {
        "out": nc.dram_tensor("out", [Q, D], FP32, kind="ExternalOutput").ap(),
    }
    with tile.TileContext(nc) as tc:
        build_kernel_body(tc, outs, ins)
    nc.compile()
    return nc


def shard_inputs(inputs):
    """Full inputs -> list of 8 per-core input dicts (numpy, contiguous)."""
    f32 = lambda a: np.ascontiguousarray(np.asarray(a), dtype=np.float32)
    pos = f32(inputs["pos"])
    key = f32(inputs["key"])
    query = f32(inputs["query"])
    value = f32(inputs["value"])
    mask = f32(inputs["key_mask"])
    shared = {
        "Wk": f32(inputs["Wk"]), "Wq": f32(inputs["Wq"]),
        "Wv": f32(inputs["Wv"]), "Wr": f32(inputs["Wr"]),
        "bk": f32(inputs["bk"]), "bq": f32(inputs["bq"]),
        "bv": f32(inputs["bv"]),
        "u": f32(inputs["u"]), "v": f32(inputs["v"]),
    }
    in_maps = []
    for c in range(NCORES):
        b, q0 = c // 4, (c % 4) * Q
        m = dict(shared)
        m["pos"] = np.ascontiguousarray(pos[b, q0:q0 + Q])
        m["key"] = key[b]
        m["value"] = value[b]
        m["query"] = np.ascontiguousarray(query[b, q0:q0 + Q])
        m["mask"] = mask[b]
        in_maps.append(m)
    return in_maps


_CACHED = {}


def kernel(**inputs):
    from concourse.bass_utils import run_bass_kernel_spmd

    if "nc" not in _CACHED:
        _CACHED["nc"] = build_program()
    nc = _CACHED["nc"]
    in_maps = shard_inputs(inputs)
    res = run_bass_kernel_spmd(nc, in_maps, core_ids=list(range(NCORES)))
    out = np.zeros((B, L, D), dtype=np.float32)
    for c in range(NCORES):
        b, q0 = c // 4, (c % 4) * Q
        out[b, q0:q0 + Q] = res.results[c]["out"]
    return out


# revision 55
# speedup vs baseline: 1.0216x; 1.0071x over previous
"""Trainium2 Bass kernel for relative-position multi-head attention.

Shapes (hardcoded): B=2, L=384, D=256, H=8, DH=32.
Sharding: 8 cores; core c handles batch b=c//4, query rows [(c%4)*96, +96).
Pure data-parallel SPMD - no collectives.

Math (per batch b, query q):
  q/k/v projections: x @ W.T + bias
  A_C[h,k] = (q_h+u_h) . k_h[k]
  B_D[h,k] = (q_h+v_h) . (Wr_h @ pos[q,k] + br_h)
           = (Wr_h^T (q_h+v_h)) . pos[q,k]   + const(h,q)   [br term is
             k-independent -> cancels in softmax -> dropped]
  score    = (A_C + B_D)/sqrt(DH) - (1-mask[k])*1e15
  out      = softmax_k(score) @ v

Key restructurings for the hardware (v2):
  * r = pos @ Wr.T (38 GFLOP) is never materialized; instead
    T[q] = Wr^T-blockdiag @ (q+v)  (a [256,8] matrix per query) and
    B_D = T^T @ posT  (1.2 GFLOP).
  * pos stays fp32 end-to-end on the DMA/cast path: the PE transposes it
    chunk-wise (float32r transpose mode), and the PSUM->SBUF copy does
    the fp32->bf16 cast on DVE/ACT.  The gpsimd engine (the v1
    bottleneck: 250us of fp32->bf16 casts) does almost nothing.
  * scores live in PSUM as [(16q x 8h) partitions, 384 k free]; per
    16-query group one psum tile gets: 2 batched A_C matmuls (block-
    diagonal (q+u) weights vs k_projT, moving N=384) + per-query B_D
    matmuls (T stationary 32 cols, posT moving N=384).
  * softmax over k (free dim): one exp per group on ACT; the key mask
    and the softmax denominator are folded into an augmented, mask-
    scaled value matrix so normalization falls out of the output matmul.
"""

import sys

for _p in ("/opt/trn_rl_repo", "/root/.axon_site/_ro/trn_rl_repo"):
    if _p not in sys.path:
        sys.path.append(_p)

import numpy as np

import concourse.bass as bass
import concourse.mybir as mybir
import concourse.tile as tile
from concourse import bacc
from concourse.masks import make_identity

FP32 = mybir.dt.float32
FP32R = mybir.dt.float32r
BF16 = mybir.dt.bfloat16

B, L, D, H = 2, 384, 256, 8
DH = D // H            # 32
Q = 96                 # queries per core
KT = L // 128          # 3 k-tiles
CB = D // 128          # 2 contraction blocks
NCORES = 8
PG = 4                 # pairs per DMA batch
NG = Q // 16           # score groups of 16 queries
SCALE = 1.0 / np.sqrt(DH)


def build_kernel_body(tc, outs, ins):
    """Emit the per-core program. outs/ins are dicts of DRAM APs."""
    from contextlib import ExitStack
    ctx = ExitStack()
    pool = lambda **kw: ctx.enter_context(tc.tile_pool(**kw))
    nc = tc.nc
    pos = ins["pos"]          # [Q, L, D] f32
    key = ins["key"]          # [L, D]
    value = ins["value"]      # [L, D]
    query = ins["query"]      # [Q, D]
    mask = ins["mask"]        # [L]
    Wk, Wq, Wv, Wr = ins["Wk"], ins["Wq"], ins["Wv"], ins["Wr"]   # [D, D]
    bk, bq, bv = ins["bk"], ins["bq"], ins["bv"]                  # [D]
    u_in, v_in = ins["u"], ins["v"]                               # [H, DH]
    out = outs["out"]         # [Q, D] f32

    const = pool(name="const", bufs=1)
    setup = pool(name="setup", bufs=2)
    psum_big = pool(name="psum_big", bufs=2, space="PSUM")
    psum_sc = pool(name="psum_sc", bufs=2, space="PSUM")
    psum_posT = pool(name="psum_posT", bufs=3, space="PSUM")
    pair_pool = pool(name="pair", bufs=3)
    pt_pool = pool(name="pt", bufs=4)

    # ---------------- identities ----------------
    ident_f = const.tile([128, 128], FP32)
    make_identity(nc, ident_f)
    ident_b = const.tile([128, 128], BF16)
    nc.vector.tensor_copy(out=ident_b, in_=ident_f)

    # ---------------- load weights + inputs ----------------
    # One consolidated DMA per tensor; T32's dependency chain (Wq -> q-proj
    # -> qv -> T) is loaded first.  Small column loads go on the ACT HWDGE
    # queue so the Pool queue is free to start emitting pos slab DMAs.
    def load_fold(ap, rows, tg, eng=nc.sync):  # [rows, D] dram -> [128, rows//128, D]
        n = rows // 128
        t = setup.tile([128, n, D], FP32, tag=f"ld_{tg}", name=f"ld_{tg}")
        eng.dma_start(out=t, in_=ap.rearrange("(i p) c -> p i c", p=128))
        return [t[:, i, :] for i in range(n)]

    key_n = load_fold(key, L, "key")
    qry_n = setup.tile([96, D], FP32)
    nc.sync.dma_start(out=qry_n, in_=query)
    Wq_n = load_fold(Wq, D, "wq")
    # Wr as [32 dh, 8 h, 256]: per-head lhsT slices at partition base 0
    wr_t = const.tile([DH, H, D], FP32)
    nc.sync.dma_start(
        out=wr_t, in_=Wr.rearrange("(h dh) c -> dh h c", dh=DH))
    Wr_h = [wr_t[:, h, :] for h in range(H)]
    # (slab DMA gate is emitted below, after the last setup DMA)

    def col_load(ap1d, n, tag):  # [n] dram -> list of [128,1] sbuf columns
        t = const.tile([128, n // 128], FP32, tag=f"col_{tag}",
                       name=f"col_{tag}")
        nc.scalar.dma_start(
            out=t, in_=ap1d.rearrange("(i p) -> p i", p=128))
        return [t[:, i:i + 1] for i in range(n // 128)]

    bq_c = col_load(bq, D, "bq")
    u_c = col_load(u_in.rearrange("h d -> (h d)"), D, "u")
    v_c = col_load(v_in.rearrange("h d -> (h d)"), D, "v")
    bk_c = col_load(bk, D, "bk")
    bv_row = const.tile([1, D], FP32)
    nc.scalar.dma_start(out=bv_row, in_=bv.rearrange("(o d) -> o d", o=1))
    # mask columns in permuted order: mask_p[r, j] = mask[3r + j]
    mask_p = const.tile([128, KT], FP32)
    nc.scalar.dma_start(
        out=mask_p, in_=mask.rearrange("(r j) -> r j", j=KT))
    # value path loads last: v_aug is only needed by the output stage
    Wk_n = load_fold(Wk, D, "wk", eng=nc.scalar)
    val_n = load_fold(value, L, "val", eng=nc.scalar)
    Wv_n = load_fold(Wv, D, "wv", eng=nc.scalar)


    # ---------------- transpose helper (fp32, PE) ----------------
    def transpose_to(dst_tiles, src_tiles, rows, cols, tag):
        """src: list of sbuf tiles [<=128, cols] covering [rows, cols].
        dst_tiles: list of CB sbuf tiles [128, rows] covering [cols, rows]."""
        for cb in range(cols // 128):
            ps = psum_big.tile([128, 512], FP32, tag="big", name="ps_tp")
            nrt = len(src_tiles)
            for i, st in enumerate(src_tiles):
                r = st.shape[0]
                nc.tensor.matmul(
                    ps[:, i * 128:i * 128 + r],
                    st[:, cb * 128:(cb + 1) * 128],
                    ident_f[:r, :r],
                    is_transpose=True,
                    start=(i == 0), stop=(i == nrt - 1))
            nc.vector.tensor_copy(out=dst_tiles[cb], in_=ps[:, :rows])

    qryT = [setup.tile([128, Q], FP32, tag=f"qryT{i}", name=f"qryT{i}") for i in range(CB)]
    transpose_to(qryT, [qry_n], Q, D, "q")
    WqT = [setup.tile([128, D], FP32, tag=f"WqT{i}", name=f"WqT{i}") for i in range(CB)]
    transpose_to(WqT, Wq_n, D, D, "wq")
    keyT = [setup.tile([128, L], FP32, tag=f"keyT{i}", name=f"keyT{i}") for i in range(CB)]
    transpose_to(keyT, key_n, L, D, "k")
    WkT = [setup.tile([128, D], FP32, tag=f"WkT{i}", name=f"WkT{i}") for i in range(CB)]
    transpose_to(WkT, Wk_n, D, D, "wk")

    # ---------------- projections ----------------
    # All k-indexed tensors below use the permuted order k = 3r + j
    # (r = partition, j = sub-tile), matching the pos DMA layout where
    # partition r holds the 3 consecutive key rows [3r, 3r+3).  Softmax
    # and the output contraction are permutation-invariant in k as long
    # as kpT / e / v_aug / mask agree, which they do by construction.
    # q_projT [d', q] f32, then qu = +u, qv = +v (per-partition adds)
    quT, qvT = [], []
    for dt in range(2):
        ps = psum_big.tile([128, 512], FP32, tag="big", name="ps_projq")
        for cb in range(CB):
            nc.tensor.matmul(
                ps[:, :Q], WqT[cb][:, dt * 128:(dt + 1) * 128], qryT[cb],
                start=(cb == 0), stop=(cb == CB - 1))
        qp = setup.tile([128, Q], FP32, tag=f"qp{dt}", name=f"qp{dt}")
        nc.vector.tensor_scalar_add(out=qp, in0=ps[:, :Q], scalar1=bq_c[dt])
        qu = const.tile([128, Q], FP32, tag=f"qu{dt}", name=f"qu{dt}")
        nc.vector.tensor_scalar_add(out=qu, in0=qp, scalar1=u_c[dt])
        qv = const.tile([128, Q], FP32, tag=f"qv{dt}", name=f"qv{dt}")
        nc.vector.tensor_scalar_add(out=qv, in0=qp, scalar1=v_c[dt])
        quT.append(qu)
        qvT.append(qv)

    # per-head qv at partition base 0
    qv_h = [setup.tile([DH, Q], FP32, tag=f"qvh{h}", name=f"qvh{h}")
            for h in range(H)]
    for h in range(H):
        dt, r = h // 4, (h % 4) * DH
        nc.vector.tensor_copy(out=qv_h[h], in_=qvT[dt][r:r + DH, :])

    # ---------------- T32: B_D stationary weights ----------------
    # T32[cb] is [128 d, 96 q, 32 c] bf16: for query q, cols [32q, 32q+32)
    # hold T_q[d, h] at local col 8*(q%4)+h and zero elsewhere, so the
    # matmul T32_q^T @ posT_q lands on score partitions 8*(q%4)+h of the
    # query's 32-partition group.
    T32 = [const.tile([128, Q * 32], BF16, tag=f"T32_{cb}", name=f"T32_{cb}")
           for cb in range(CB)]
    for cb in range(CB):
        nc.vector.memset(T32[cb], 0.0)

    def emit_T32():
        # emitted in setup: moving it into the loop contends with the pair
        # transposes for the pt psum slots and loses ~3us
        for cb in range(CB):
            t32v = T32[cb].rearrange("p (t x) -> p t x", x=128)
            for h in range(H):
                # share the posT psum slots so the 16 T matmuls pipeline
                # instead of chasing 2 "big" slots
                ps = psum_posT.tile([128, 512], FP32, tag="pt", name="ps_T")
                nc.tensor.matmul(
                    ps[:, :Q], Wr_h[h][:, cb * 128:(cb + 1) * 128],
                    qv_h[h], start=True, stop=True)
                # dst cols 128t + 40j + h over (t, j): stride-40 step slice
                if h % 2 == 0:
                    nc.vector.tensor_copy(
                        out=t32v[:, :, h::40],
                        in_=ps[:, :Q].rearrange("p (t j) -> p t j", j=4))
                else:
                    nc.scalar.activation(
                        out=t32v[:, :, h::40],
                        in_=ps[:, :Q].rearrange("p (t j) -> p t j", j=4),
                        func=mybir.ActivationFunctionType.Copy)

    emit_T32()

    # ---------------- qu_bd: batched A_C stationary weights ----------
    # qu_bd[cb] is [128 d', 6 g, 128 (4J,4j,8h)] bf16: col (g,J,j,h) holds
    # (q+u)[d', q=16g+4J+j] on head h's 32-row diagonal block, 0 elsewhere.
    qu_bd = [const.tile([128, NG * 128], BF16, tag=f"qbd{cb}",
                        name=f"qbd{cb}") for cb in range(CB)]
    for cb in range(CB):
        nc.vector.memset(qu_bd[cb], 0.0)
        qbv = qu_bd[cb].rearrange(
            "p (g J j x) -> p g J j x", J=4, j=4, x=H)
        for hl in range(4):
            h = cb * 4 + hl
            nc.vector.tensor_copy(
                out=qbv[hl * DH:(hl + 1) * DH, :, :, :, h],
                in_=quT[cb][hl * DH:(hl + 1) * DH, :].rearrange(
                    "p (g J j) -> p g J j", J=4, j=4))

    # k_projT [d', k] bf16, full [128, L] tiles (contraction layout for
    # A_C) -- emitted inside the main loop after group 0's transposes, so
    # the PE starts on pos data as soon as the first slab lands.
    kpT = [setup.tile([128, L], BF16, tag=f"kpT{i}", name=f"kpT{i}")
           for i in range(CB)]

    def emit_kpT():
        for dt in range(2):
            ps = psum_big.tile([128, 512], FP32, tag="big", name="ps_proj")
            for cb in range(CB):
                nc.tensor.matmul(
                    ps[:, :L], WkT[cb][:, dt * 128:(dt + 1) * 128],
                    keyT[cb].rearrange("p (r j) -> p j r", j=KT),
                    start=(cb == 0), stop=(cb == CB - 1))
            nc.vector.tensor_scalar_add(
                out=kpT[dt], in0=ps[:, :L], scalar1=bk_c[dt])

    # v_proj natural [k, d'] + ones column per head -> v_aug [128, H, DH+1]
    # bf16; rows scaled by key mask (folds both the -1e15 mask bias and the
    # softmax denominator's mask into the output matmul).  Only the output
    # stage needs it, so it is emitted mid-loop after group 0.
    ones_1 = const.tile([1, D], FP32)
    nc.vector.memset(ones_1, 1.0)
    valT = [setup.tile([128, L], FP32, tag=f"valT{i}", name=f"valT{i}")
            for i in range(CB)]
    WvT = [setup.tile([128, D], FP32, tag=f"WvT{i}", name=f"WvT{i}")
           for i in range(CB)]
    v_aug = [const.tile([128, H, DH + 1], BF16, tag=f"va{j}", name=f"va{j}")
             for j in range(KT)]

    def emit_vpath():
        transpose_to(valT, val_n, L, D, "v")
        transpose_to(WvT, Wv_n, D, D, "wv")
        for j in range(KT):
            ps = psum_big.tile([128, 512], FP32, tag="big", name="ps_projv")
            for cb in range(CB):
                nc.tensor.matmul(
                    ps[:, :D],
                    valT[cb].rearrange("p (r j) -> p j r", j=KT)[:, j],
                    WvT[cb],
                    start=(cb == 0), stop=False)
            # + bias bv broadcast over rows (rank-1 matmul with ones lhsT)
            nc.tensor.matmul(ps[:, :D], ones_1[:, :128], bv_row,
                             start=False, stop=True)
            va = v_aug[j]
            nc.vector.memset(va, 1.0)
            nc.vector.tensor_copy(
                out=va[:, :, 0:DH],
                in_=ps[:, :D].rearrange("p (h d) -> p h d", h=H))
            nc.vector.tensor_scalar_mul(
                out=va, in0=va, scalar1=mask_p[:, j:j + 1])

    # ---------------- eT: exp(scores) transposed, [k, (h,q)] ----------
    eT = [const.tile([128, H * Q], BF16, tag=f"eT{kt}", name=f"eT{kt}")
          for kt in range(KT)]

    # ---------------- main loop: score groups of 16 queries ------------
    # Per group: stream 16 pairs of transposes + PSUM->SBUF copies first
    # (PE never waits on DVE/ACT), then run the batched A_C + 32 B_D
    # matmuls over the buffered pT tiles, then exp + e-transpose.
    slab = [None]

    def emit_transposes(q):
        """Transpose pair q's pos slab; returns the 2 buffered pT tiles."""
        i = q % PG
        pTs = []
        for cb in range(CB):
            ps = psum_posT.tile([128, 1024], BF16, tag="pt", name="pt_ps")
            for j in range(KT):
                nc.tensor.matmul(
                    ps[:, j * 128:(j + 1) * 128],
                    slab[0][:, i, j, cb * 128:(cb + 1) * 128],
                    ident_b,
                    is_transpose=True,
                    start=(j == 0), stop=(j == KT - 1))
            pT = pt_pool.tile([128, L], BF16, tag=f"posT{cb}",
                              name=f"posT{cb}", bufs=22)
            if cb == 0:
                nc.vector.tensor_copy(out=pT, in_=ps[:, :L])
            else:
                nc.scalar.activation(
                    out=pT, in_=ps[:, :L],
                    func=mybir.ActivationFunctionType.Copy)
            pTs.append(pT)
        return pTs

    def emit_eT(g, e):
        """e-transpose for a finished score group (deferred one group so
        PE never waits on the exp)."""
        for kt in range(KT):
            pe = psum_big.tile([128, 1024], BF16, tag="eTp", name="pe",
                               bufs=1)
            nc.tensor.matmul(
                pe[:, :128], e[:, kt * 128:(kt + 1) * 128], ident_b,
                is_transpose=True, start=True, stop=True)
            dst = eT[kt].rearrange(
                "p (h g J j) -> p g J j h", h=H, g=NG, J=4, j=4)[:, g]
            nc.vector.tensor_copy(
                out=dst,
                in_=pe[:, :128].rearrange("p (J j h) -> p J j h", J=4, j=4))

    pending_eT = None
    for g in range(NG):
        pT_buf = []
        for jj in range(16):
            q = g * 16 + jj
            if q % PG == 0:
                # SWDGE cast-DMA: fp32 HBM -> bf16 SBUF, 3KB-contiguous
                # descriptors (partition r holds key rows [3r, 3r+3)).
                s = pair_pool.tile([128, PG, KT, D], BF16, tag="slab",
                                   name="slab", bufs=10)
                if q < 10 * PG:
                    # WAW-gate the pre-loop slabs behind the last critical
                    # sync-ring DMA, so these big transfers don't starve
                    # the small setup loads on the shared SDMA engines
                    # (the scheduler would otherwise hoist them).
                    nc.gpsimd.tensor_copy(
                        out=s[0:1, 0, 0, 4:8], in_=key_n[2][0:1, 0:4])
                nc.gpsimd.dma_start(
                    out=s,
                    in_=pos[q:q + PG].rearrange(
                        "g (r j) c -> r g j c", j=KT))
                slab[0] = s
            pT_buf.append(emit_transposes(q))
        if g == 0:
            emit_kpT()
        elif g == 1:
            emit_vpath()
        if pending_eT is not None:
            emit_eT(*pending_eT)
        sc = psum_sc.tile([128, 512], FP32, tag="sc", name=f"sc{g}")
        scv = sc[:, :L]
        for cb in range(CB):
            nc.tensor.matmul(
                scv, qu_bd[cb][:, g * 128:(g + 1) * 128], kpT[cb],
                start=(cb == 0), stop=(cb == CB - 1))
        # J-round-robin order: consecutive matmuls land on different
        # 32-partition column groups of the PE array, so each LDWEIGHTS
        # overlaps the previous matmul (col-tiling concurrency).
        for jj in [0, 4, 8, 12, 1, 5, 9, 13, 2, 6, 10, 14, 3, 7, 11, 15]:
            q = g * 16 + jj
            J = jj // 4
            for cb in range(CB):
                # start/stop bookkeeping lives on the A_C matmuls (which
                # cover all 128 partitions); on HW stop is a no-op and
                # accumulation is per-element, so skip the group check.
                nc.tensor.matmul(
                    scv[J * 32:(J + 1) * 32, :],
                    T32[cb][:, q * 32:(q + 1) * 32],
                    pT_buf[jj][cb],
                    start=False, stop=False, skip_group_check=True,
                    tile_position=(0, J * 32))
        e = pair_pool.tile([128, L], BF16, tag="e", name=f"e{g}")
        nc.scalar.activation(
            out=e, in_=scv, func=mybir.ActivationFunctionType.Exp,
            scale=float(SCALE))
        pending_eT = (g, e)
    emit_eT(*pending_eT)

    # ---------------- output matmuls + normalize ----------------
    # Batched phases with 4 heads packed per PSUM bank, so the 8 heads
    # don't serialize through a single po -> copy -> transpose -> recip
    # latency chain.
    out_sb = setup.tile([96, D], FP32, tag="osb")
    tmp4 = [pt_pool.tile([DH + 1, 4 * Q], FP32, tag=f"otmp{t}",
                         name=f"otmp{t}") for t in range(2)]
    pot4 = []
    for t in range(2):
        po = psum_big.tile([DH + 1, 512], FP32, tag="big", name=f"po{t}")
        for hh in range(4):
            h = t * 4 + hh
            for kt in range(KT):
                nc.tensor.matmul(
                    po[:, hh * Q:(hh + 1) * Q],
                    v_aug[kt][:, h, :], eT[kt][:, h * Q:(h + 1) * Q],
                    start=(hh == 0 and kt == 0),
                    stop=(hh == 3 and kt == KT - 1),
                    skip_group_check=True)
        nc.vector.tensor_copy(out=tmp4[t], in_=po[:, :4 * Q])
        pot = psum_posT.tile([Q, 512], FP32, tag="pt", name=f"pot{t}")
        for hh in range(4):
            nc.tensor.matmul(
                pot[:, hh * (DH + 1):(hh + 1) * (DH + 1)],
                tmp4[t][:, hh * Q:(hh + 1) * Q],
                ident_f[:DH + 1, :DH + 1],
                is_transpose=True,
                start=(hh == 0), stop=(hh == 3), skip_group_check=True)
        pot4.append(pot)
    for t in range(2):
        rec = pt_pool.tile([Q, 4], FP32, tag="rec", name=f"rec{t}")
        nc.vector.reciprocal(
            out=rec,
            in_=pot4[t][:, :4 * (DH + 1)].rearrange(
                "p (hh x) -> p hh x", x=DH + 1)[:, :, DH])
        for hh in range(4):
            h = t * 4 + hh
            nc.vector.tensor_scalar_mul(
                out=out_sb[:, h * DH:(h + 1) * DH],
                in0=pot4[t][:, hh * (DH + 1):hh * (DH + 1) + DH],
                scalar1=rec[:, hh:hh + 1])

    nc.sync.dma_start(out=out, in_=out_sb)
    ctx.close()


def build_program():
    nc = bacc.Bacc(
        "TRN2", target_bir_lowering=False, debug=False,
        num_devices=NCORES)
    ins = {
        "pos": nc.dram_tensor("pos", [Q, L, D], FP32, kind="ExternalInput").ap(),
        "key": nc.dram_tensor("key", [L, D], FP32, kind="ExternalInput").ap(),
        "value": nc.dram_tensor("value", [L, D], FP32, kind="ExternalInput").ap(),
        "query": nc.dram_tensor("query", [Q, D], FP32, kind="ExternalInput").ap(),
        "mask": nc.dram_tensor("mask", [L], FP32, kind="ExternalInput").ap(),
        "Wk": nc.dram_tensor("Wk", [D, D], FP32, kind="ExternalInput").ap(),
        "Wq": nc.dram_tensor("Wq", [D, D], FP32, kind="ExternalInput").ap(),
        "Wv": nc.dram_tensor("Wv", [D, D], FP32, kind="ExternalInput").ap(),
        "Wr": nc.dram_tensor("Wr", [D, D], FP32, kind="ExternalInput").ap(),
        "bk": nc.dram_tensor("bk", [D], FP32, kind="ExternalInput").ap(),
        "bq": nc.dram_tensor("bq", [D], FP32, kind="ExternalInput").ap(),
        "bv": nc.dram_tensor("bv", [D], FP32, kind="ExternalInput").ap(),
        "u": nc.dram_tensor("u", [H, DH], FP32, kind="ExternalInput").ap(),
        "v": nc.dram_tensor("v", [H, DH], FP32, kind="ExternalInput").ap(),
    }
    outs = {
        "out": nc.dram_tensor("out", [Q, D], FP32, kind="ExternalOutput").ap(),
    }
    with tile.TileContext(nc) as tc:
        build_kernel_body(tc, outs, ins)
    nc.compile()
    return nc


def shard_inputs(inputs):
    """Full inputs -> list of 8 per-core input dicts (numpy, contiguous)."""
    f32 = lambda a: np.ascontiguousarray(np.asarray(a), dtype=np.float32)
    pos = f32(inputs["pos"])
    key = f32(inputs["key"])
    query = f32(inputs["query"])
    value = f32(inputs["value"])
    mask = f32(inputs["key_mask"])
    shared = {
        "Wk": f32(inputs["Wk"]), "Wq": f32(inputs["Wq"]),
        "Wv": f32(inputs["Wv"]), "Wr": f32(inputs["Wr"]),
        "bk": f32(inputs["bk"]), "bq": f32(inputs["bq"]),
        "bv": f32(inputs["bv"]),
        "u": f32(inputs["u"]), "v": f32(inputs["v"]),
    }
    in_maps = []
    for c in range(NCORES):
        b, q0 = c // 4, (c % 4) * Q
        m = dict(shared)
        m["pos"] = np.ascontiguousarray(pos[b, q0:q0 + Q])
        m["key"] = key[b]
        m["value"] = value[b]
        m["query"] = np.ascontiguousarray(query[b, q0:q0 + Q])
        m["mask"] = mask[b]
        in_maps.append(m)
    return in_maps


_CACHED = {}


def kernel(**inputs):
    from concourse.bass_utils import run_bass_kernel_spmd

    if "nc" not in _CACHED:
        _CACHED["nc"] = build_program()
    nc = _CACHED["nc"]
    in_maps = shard_inputs(inputs)
    res = run_bass_kernel_spmd(nc, in_maps, core_ids=list(range(NCORES)))
    out = np.zeros((B, L, D), dtype=np.float32)
    for c in range(NCORES):
        b, q0 = c // 4, (c % 4) * Q
        out[b, q0:q0 + Q] = res.results[c]["out"]
    return out


# revision 56
# speedup vs baseline: 1.0992x; 1.0760x over previous
"""Trainium2 Bass kernel for relative-position multi-head attention.

Shapes (hardcoded): B=2, L=384, D=256, H=8, DH=32.
Sharding: 8 cores; core c handles batch b=c//4, query rows [(c%4)*96, +96).
Pure data-parallel SPMD - no collectives.

Math (per batch b, query q):
  q/k/v projections: x @ W.T + bias
  A_C[h,k] = (q_h+u_h) . k_h[k]
  B_D[h,k] = (q_h+v_h) . (Wr_h @ pos[q,k] + br_h)
           = (Wr_h^T (q_h+v_h)) . pos[q,k]   + const(h,q)   [br term is
             k-independent -> cancels in softmax -> dropped]
  score    = (A_C + B_D)/sqrt(DH) - (1-mask[k])*1e15
  out      = softmax_k(score) @ v

Key restructurings for the hardware (v2):
  * r = pos @ Wr.T (38 GFLOP) is never materialized; instead
    T[q] = Wr^T-blockdiag @ (q+v)  (a [256,8] matrix per query) and
    B_D = T^T @ posT  (1.2 GFLOP).
  * pos stays fp32 end-to-end on the DMA/cast path: the PE transposes it
    chunk-wise (float32r transpose mode), and the PSUM->SBUF copy does
    the fp32->bf16 cast on DVE/ACT.  The gpsimd engine (the v1
    bottleneck: 250us of fp32->bf16 casts) does almost nothing.
  * scores live in PSUM as [(16q x 8h) partitions, 384 k free]; per
    16-query group one psum tile gets: 2 batched A_C matmuls (block-
    diagonal (q+u) weights vs k_projT, moving N=384) + per-query B_D
    matmuls (T stationary 32 cols, posT moving N=384).
  * softmax over k (free dim): one exp per group on ACT; the key mask
    and the softmax denominator are folded into an augmented, mask-
    scaled value matrix so normalization falls out of the output matmul.
"""

import sys

for _p in ("/opt/trn_rl_repo", "/root/.axon_site/_ro/trn_rl_repo"):
    if _p not in sys.path:
        sys.path.append(_p)

import numpy as np

import concourse.bass as bass
import concourse.mybir as mybir
import concourse.tile as tile
from concourse import bacc
from concourse.masks import make_identity

FP32 = mybir.dt.float32
FP32R = mybir.dt.float32r
BF16 = mybir.dt.bfloat16

B, L, D, H = 2, 384, 256, 8
DH = D // H            # 32
Q = 96                 # queries per core
KT = L // 128          # 3 k-tiles
CB = D // 128          # 2 contraction blocks
NCORES = 8
PG = 8                 # pairs per DMA batch
NG = Q // 16           # score groups of 16 queries
SCALE = 1.0 / np.sqrt(DH)


def build_kernel_body(tc, outs, ins):
    """Emit the per-core program. outs/ins are dicts of DRAM APs."""
    from contextlib import ExitStack
    ctx = ExitStack()
    pool = lambda **kw: ctx.enter_context(tc.tile_pool(**kw))
    nc = tc.nc
    pos = ins["pos"]          # [Q, L, D] f32
    key = ins["key"]          # [L, D]
    value = ins["value"]      # [L, D]
    query = ins["query"]      # [Q, D]
    mask = ins["mask"]        # [L]
    Wk, Wq, Wv, Wr = ins["Wk"], ins["Wq"], ins["Wv"], ins["Wr"]   # [D, D]
    bk, bq, bv = ins["bk"], ins["bq"], ins["bv"]                  # [D]
    u_in, v_in = ins["u"], ins["v"]                               # [H, DH]
    out = outs["out"]         # [Q, D] f32

    const = pool(name="const", bufs=1)
    setup = pool(name="setup", bufs=2)
    psum_big = pool(name="psum_big", bufs=2, space="PSUM")
    psum_sc = pool(name="psum_sc", bufs=2, space="PSUM")
    psum_posT = pool(name="psum_posT", bufs=3, space="PSUM")
    pair_pool = pool(name="pair", bufs=3)
    pt_pool = pool(name="pt", bufs=4)

    # ---------------- identities ----------------
    ident_f = const.tile([128, 128], FP32)
    make_identity(nc, ident_f)
    ident_b = const.tile([128, 128], BF16)
    nc.vector.tensor_copy(out=ident_b, in_=ident_f)

    # ---------------- load weights + inputs ----------------
    # One consolidated DMA per tensor; T32's dependency chain (Wq -> q-proj
    # -> qv -> T) is loaded first.  Small column loads go on the ACT HWDGE
    # queue so the Pool queue is free to start emitting pos slab DMAs.
    def load_fold(ap, rows, tg, eng=nc.sync):  # [rows, D] dram -> [128, rows//128, D]
        n = rows // 128
        t = setup.tile([128, n, D], FP32, tag=f"ld_{tg}", name=f"ld_{tg}")
        eng.dma_start(out=t, in_=ap.rearrange("(i p) c -> p i c", p=128))
        return [t[:, i, :] for i in range(n)]

    key_n = load_fold(key, L, "key")
    qry_n = setup.tile([96, D], FP32)
    nc.sync.dma_start(out=qry_n, in_=query)
    Wq_n = load_fold(Wq, D, "wq")
    # Wr as [32 dh, 8 h, 256]: per-head lhsT slices at partition base 0
    wr_t = const.tile([DH, H, D], FP32)
    nc.sync.dma_start(
        out=wr_t, in_=Wr.rearrange("(h dh) c -> dh h c", dh=DH))
    Wr_h = [wr_t[:, h, :] for h in range(H)]
    # (slab DMA gate is emitted below, after the last setup DMA)

    def col_load(ap1d, n, tag):  # [n] dram -> list of [128,1] sbuf columns
        t = const.tile([128, n // 128], FP32, tag=f"col_{tag}",
                       name=f"col_{tag}")
        nc.scalar.dma_start(
            out=t, in_=ap1d.rearrange("(i p) -> p i", p=128))
        return [t[:, i:i + 1] for i in range(n // 128)]

    bq_c = col_load(bq, D, "bq")
    u_c = col_load(u_in.rearrange("h d -> (h d)"), D, "u")
    v_c = col_load(v_in.rearrange("h d -> (h d)"), D, "v")
    bk_c = col_load(bk, D, "bk")
    bv_row = const.tile([1, D], FP32)
    nc.scalar.dma_start(out=bv_row, in_=bv.rearrange("(o d) -> o d", o=1))
    # mask columns in permuted order: mask_p[r, j] = mask[3r + j]
    mask_p = const.tile([128, KT], FP32)
    nc.scalar.dma_start(
        out=mask_p, in_=mask.rearrange("(r j) -> r j", j=KT))
    # value path loads last: v_aug is only needed by the output stage
    Wk_n = load_fold(Wk, D, "wk", eng=nc.scalar)
    val_n = load_fold(value, L, "val", eng=nc.scalar)
    Wv_n = load_fold(Wv, D, "wv", eng=nc.scalar)


    # ---------------- transpose helper (fp32, PE) ----------------
    def transpose_to(dst_tiles, src_tiles, rows, cols, tag):
        """src: list of sbuf tiles [<=128, cols] covering [rows, cols].
        dst_tiles: list of CB sbuf tiles [128, rows] covering [cols, rows]."""
        for cb in range(cols // 128):
            ps = psum_big.tile([128, 512], FP32, tag="big", name="ps_tp")
            nrt = len(src_tiles)
            for i, st in enumerate(src_tiles):
                r = st.shape[0]
                nc.tensor.matmul(
                    ps[:, i * 128:i * 128 + r],
                    st[:, cb * 128:(cb + 1) * 128],
                    ident_f[:r, :r],
                    is_transpose=True,
                    start=(i == 0), stop=(i == nrt - 1))
            nc.vector.tensor_copy(out=dst_tiles[cb], in_=ps[:, :rows])

    qryT = [setup.tile([128, Q], FP32, tag=f"qryT{i}", name=f"qryT{i}") for i in range(CB)]
    transpose_to(qryT, [qry_n], Q, D, "q")
    WqT = [setup.tile([128, D], FP32, tag=f"WqT{i}", name=f"WqT{i}") for i in range(CB)]
    transpose_to(WqT, Wq_n, D, D, "wq")
    keyT = [setup.tile([128, L], FP32, tag=f"keyT{i}", name=f"keyT{i}") for i in range(CB)]
    transpose_to(keyT, key_n, L, D, "k")
    WkT = [setup.tile([128, D], FP32, tag=f"WkT{i}", name=f"WkT{i}") for i in range(CB)]
    transpose_to(WkT, Wk_n, D, D, "wk")

    # ---------------- projections ----------------
    # All k-indexed tensors below use the permuted order k = 3r + j
    # (r = partition, j = sub-tile), matching the pos DMA layout where
    # partition r holds the 3 consecutive key rows [3r, 3r+3).  Softmax
    # and the output contraction are permutation-invariant in k as long
    # as kpT / e / v_aug / mask agree, which they do by construction.
    # q_projT [d', q] f32, then qu = +u, qv = +v (per-partition adds)
    quT, qvT = [], []
    for dt in range(2):
        ps = psum_big.tile([128, 512], FP32, tag="big", name="ps_projq")
        for cb in range(CB):
            nc.tensor.matmul(
                ps[:, :Q], WqT[cb][:, dt * 128:(dt + 1) * 128], qryT[cb],
                start=(cb == 0), stop=(cb == CB - 1))
        qp = setup.tile([128, Q], FP32, tag=f"qp{dt}", name=f"qp{dt}")
        nc.vector.tensor_scalar_add(out=qp, in0=ps[:, :Q], scalar1=bq_c[dt])
        qu = const.tile([128, Q], FP32, tag=f"qu{dt}", name=f"qu{dt}")
        nc.vector.tensor_scalar_add(out=qu, in0=qp, scalar1=u_c[dt])
        qv = const.tile([128, Q], FP32, tag=f"qv{dt}", name=f"qv{dt}")
        nc.vector.tensor_scalar_add(out=qv, in0=qp, scalar1=v_c[dt])
        quT.append(qu)
        qvT.append(qv)

    # per-head qv at partition base 0
    qv_h = [setup.tile([DH, Q], FP32, tag=f"qvh{h}", name=f"qvh{h}")
            for h in range(H)]
    for h in range(H):
        dt, r = h // 4, (h % 4) * DH
        nc.vector.tensor_copy(out=qv_h[h], in_=qvT[dt][r:r + DH, :])

    # ---------------- T32: B_D stationary weights ----------------
    # T32[cb] is [128 d, 96 q, 32 c] bf16: for query q, cols [32q, 32q+32)
    # hold T_q[d, h] at local col 8*(q%4)+h and zero elsewhere, so the
    # matmul T32_q^T @ posT_q lands on score partitions 8*(q%4)+h of the
    # query's 32-partition group.
    T32 = [const.tile([128, Q * 32], BF16, tag=f"T32_{cb}", name=f"T32_{cb}")
           for cb in range(CB)]
    for cb in range(CB):
        nc.vector.memset(T32[cb], 0.0)

    def emit_T32():
        # emitted in setup: moving it into the loop contends with the pair
        # transposes for the pt psum slots and loses ~3us
        for cb in range(CB):
            t32v = T32[cb].rearrange("p (t x) -> p t x", x=128)
            for h in range(H):
                # share the posT psum slots so the 16 T matmuls pipeline
                # instead of chasing 2 "big" slots
                ps = psum_posT.tile([128, 512], FP32, tag="pt", name="ps_T")
                nc.tensor.matmul(
                    ps[:, :Q], Wr_h[h][:, cb * 128:(cb + 1) * 128],
                    qv_h[h], start=True, stop=True)
                # dst cols 128t + 40j + h over (t, j): stride-40 step slice
                if h % 2 == 0:
                    nc.vector.tensor_copy(
                        out=t32v[:, :, h::40],
                        in_=ps[:, :Q].rearrange("p (t j) -> p t j", j=4))
                else:
                    nc.scalar.activation(
                        out=t32v[:, :, h::40],
                        in_=ps[:, :Q].rearrange("p (t j) -> p t j", j=4),
                        func=mybir.ActivationFunctionType.Copy)

    emit_T32()

    # ---------------- qu_bd: batched A_C stationary weights ----------
    # qu_bd[cb] is [128 d', 6 g, 128 (4J,4j,8h)] bf16: col (g,J,j,h) holds
    # (q+u)[d', q=16g+4J+j] on head h's 32-row diagonal block, 0 elsewhere.
    qu_bd = [const.tile([128, NG * 128], BF16, tag=f"qbd{cb}",
                        name=f"qbd{cb}") for cb in range(CB)]
    for cb in range(CB):
        nc.vector.memset(qu_bd[cb], 0.0)
        qbv = qu_bd[cb].rearrange(
            "p (g J j x) -> p g J j x", J=4, j=4, x=H)
        for hl in range(4):
            h = cb * 4 + hl
            nc.vector.tensor_copy(
                out=qbv[hl * DH:(hl + 1) * DH, :, :, :, h],
                in_=quT[cb][hl * DH:(hl + 1) * DH, :].rearrange(
                    "p (g J j) -> p g J j", J=4, j=4))

    # k_projT [d', k] bf16, full [128, L] tiles (contraction layout for
    # A_C) -- emitted inside the main loop after group 0's transposes, so
    # the PE starts on pos data as soon as the first slab lands.
    kpT = [setup.tile([128, L], BF16, tag=f"kpT{i}", name=f"kpT{i}")
           for i in range(CB)]

    def emit_kpT():
        for dt in range(2):
            ps = psum_big.tile([128, 512], FP32, tag="big", name="ps_proj")
            for cb in range(CB):
                nc.tensor.matmul(
                    ps[:, :L], WkT[cb][:, dt * 128:(dt + 1) * 128],
                    keyT[cb].rearrange("p (r j) -> p j r", j=KT),
                    start=(cb == 0), stop=(cb == CB - 1))
            nc.vector.tensor_scalar_add(
                out=kpT[dt], in0=ps[:, :L], scalar1=bk_c[dt])

    # v_proj natural [k, d'] + ones column per head -> v_aug [128, H, DH+1]
    # bf16; rows scaled by key mask (folds both the -1e15 mask bias and the
    # softmax denominator's mask into the output matmul).  Only the output
    # stage needs it, so it is emitted mid-loop after group 0.
    ones_1 = const.tile([1, D], FP32)
    nc.vector.memset(ones_1, 1.0)
    valT = [setup.tile([128, L], FP32, tag=f"valT{i}", name=f"valT{i}")
            for i in range(CB)]
    WvT = [setup.tile([128, D], FP32, tag=f"WvT{i}", name=f"WvT{i}")
           for i in range(CB)]
    v_aug = [const.tile([128, H, DH + 1], BF16, tag=f"va{j}", name=f"va{j}")
             for j in range(KT)]

    def emit_vpath():
        transpose_to(valT, val_n, L, D, "v")
        transpose_to(WvT, Wv_n, D, D, "wv")
        for j in range(KT):
            ps = psum_big.tile([128, 512], FP32, tag="big", name="ps_projv")
            for cb in range(CB):
                nc.tensor.matmul(
                    ps[:, :D],
                    valT[cb].rearrange("p (r j) -> p j r", j=KT)[:, j],
                    WvT[cb],
                    start=(cb == 0), stop=False)
            # + bias bv broadcast over rows (rank-1 matmul with ones lhsT)
            nc.tensor.matmul(ps[:, :D], ones_1[:, :128], bv_row,
                             start=False, stop=True)
            va = v_aug[j]
            nc.vector.memset(va, 1.0)
            nc.vector.tensor_copy(
                out=va[:, :, 0:DH],
                in_=ps[:, :D].rearrange("p (h d) -> p h d", h=H))
            nc.vector.tensor_scalar_mul(
                out=va, in0=va, scalar1=mask_p[:, j:j + 1])

    # ---------------- eT: exp(scores) transposed, [k, (h,q)] ----------
    eT = [const.tile([128, H * Q], BF16, tag=f"eT{kt}", name=f"eT{kt}")
          for kt in range(KT)]

    # ---------------- main loop: score groups of 16 queries ------------
    # Per group: stream 16 pairs of transposes + PSUM->SBUF copies first
    # (PE never waits on DVE/ACT), then run the batched A_C + 32 B_D
    # matmuls over the buffered pT tiles, then exp + e-transpose.
    slab = [None]

    def emit_transposes(q):
        """Transpose pair q's pos slab; returns the 2 buffered pT tiles."""
        i = q % PG
        pTs = []
        for cb in range(CB):
            ps = psum_posT.tile([128, 1024], BF16, tag="pt", name="pt_ps")
            for j in range(KT):
                nc.tensor.matmul(
                    ps[:, j * 128:(j + 1) * 128],
                    slab[0][:, i, j, cb * 128:(cb + 1) * 128],
                    ident_b,
                    is_transpose=True,
                    start=(j == 0), stop=(j == KT - 1))
            pT = pt_pool.tile([128, L], BF16, tag=f"posT{cb}",
                              name=f"posT{cb}", bufs=22)
            if cb == 0:
                nc.vector.tensor_copy(out=pT, in_=ps[:, :L])
            else:
                nc.scalar.activation(
                    out=pT, in_=ps[:, :L],
                    func=mybir.ActivationFunctionType.Copy)
            pTs.append(pT)
        return pTs

    def emit_eT(g, e):
        """e-transpose for a finished score group (deferred one group so
        PE never waits on the exp)."""
        for kt in range(KT):
            pe = psum_big.tile([128, 1024], BF16, tag="eTp", name="pe",
                               bufs=1)
            nc.tensor.matmul(
                pe[:, :128], e[:, kt * 128:(kt + 1) * 128], ident_b,
                is_transpose=True, start=True, stop=True)
            dst = eT[kt].rearrange(
                "p (h g J j) -> p g J j h", h=H, g=NG, J=4, j=4)[:, g]
            nc.vector.tensor_copy(
                out=dst,
                in_=pe[:, :128].rearrange("p (J j h) -> p J j h", J=4, j=4))

    pending_eT = None
    for g in range(NG):
        pT_buf = []
        for jj in range(16):
            q = g * 16 + jj
            if q % PG == 0:
                # SWDGE cast-DMA: fp32 HBM -> bf16 SBUF, 3KB-contiguous
                # descriptors (partition r holds key rows [3r, 3r+3)).
                s = pair_pool.tile([128, PG, KT, D], BF16, tag="slab",
                                   name="slab", bufs=5)
                if q < 5 * PG:
                    # WAW-gate the pre-loop slabs behind the last critical
                    # sync-ring DMA, so these big transfers don't starve
                    # the small setup loads on the shared SDMA engines
                    # (the scheduler would otherwise hoist them).
                    nc.gpsimd.tensor_copy(
                        out=s[0:1, 0, 0, 4:8], in_=key_n[2][0:1, 0:4])
                nc.gpsimd.dma_start(
                    out=s,
                    in_=pos[q:q + PG].rearrange(
                        "g (r j) c -> r g j c", j=KT))
                slab[0] = s
            pT_buf.append(emit_transposes(q))
        if g == 0:
            emit_kpT()
        elif g == 1:
            emit_vpath()
        if pending_eT is not None:
            emit_eT(*pending_eT)
        sc = psum_sc.tile([128, 512], FP32, tag="sc", name=f"sc{g}")
        scv = sc[:, :L]
        for cb in range(CB):
            nc.tensor.matmul(
                scv, qu_bd[cb][:, g * 128:(g + 1) * 128], kpT[cb],
                start=(cb == 0), stop=(cb == CB - 1))
        # J-round-robin order: consecutive matmuls land on different
        # 32-partition column groups of the PE array, so each LDWEIGHTS
        # overlaps the previous matmul (col-tiling concurrency).
        for jj in [0, 4, 8, 12, 1, 5, 9, 13, 2, 6, 10, 14, 3, 7, 11, 15]:
            q = g * 16 + jj
            J = jj // 4
            for cb in range(CB):
                # start/stop bookkeeping lives on the A_C matmuls (which
                # cover all 128 partitions); on HW stop is a no-op and
                # accumulation is per-element, so skip the group check.
                nc.tensor.matmul(
                    scv[J * 32:(J + 1) * 32, :],
                    T32[cb][:, q * 32:(q + 1) * 32],
                    pT_buf[jj][cb],
                    start=False, stop=False, skip_group_check=True,
                    tile_position=(0, J * 32))
        e = pair_pool.tile([128, L], BF16, tag="e", name=f"e{g}")
        nc.scalar.activation(
            out=e, in_=scv, func=mybir.ActivationFunctionType.Exp,
            scale=float(SCALE))
        pending_eT = (g, e)
    emit_eT(*pending_eT)

    # ---------------- output matmuls + normalize ----------------
    # Batched phases with 4 heads packed per PSUM bank, so the 8 heads
    # don't serialize through a single po -> copy -> transpose -> recip
    # latency chain.
    out_sb = setup.tile([96, D], FP32, tag="osb")
    tmp4 = [pt_pool.tile([DH + 1, 4 * Q], FP32, tag=f"otmp{t}",
                         name=f"otmp{t}") for t in range(2)]
    pot4 = []
    for t in range(2):
        po = psum_big.tile([DH + 1, 512], FP32, tag="big", name=f"po{t}")
        for hh in range(4):
            h = t * 4 + hh
            for kt in range(KT):
                nc.tensor.matmul(
                    po[:, hh * Q:(hh + 1) * Q],
                    v_aug[kt][:, h, :], eT[kt][:, h * Q:(h + 1) * Q],
                    start=(hh == 0 and kt == 0),
                    stop=(hh == 3 and kt == KT - 1),
                    skip_group_check=True)
        nc.vector.tensor_copy(out=tmp4[t], in_=po[:, :4 * Q])
        pot = psum_posT.tile([Q, 512], FP32, tag="pt", name=f"pot{t}")
        for hh in range(4):
            nc.tensor.matmul(
                pot[:, hh * (DH + 1):(hh + 1) * (DH + 1)],
                tmp4[t][:, hh * Q:(hh + 1) * Q],
                ident_f[:DH + 1, :DH + 1],
                is_transpose=True,
                start=(hh == 0), stop=(hh == 3), skip_group_check=True)
        pot4.append(pot)
    for t in range(2):
        rec = pt_pool.tile([Q, 4], FP32, tag="rec", name=f"rec{t}")
        nc.vector.reciprocal(
            out=rec,
            in_=pot4[t][:, :4 * (DH + 1)].rearrange(
                "p (hh x) -> p hh x", x=DH + 1)[:, :, DH])
        for hh in range(4):
            h = t * 4 + hh
            nc.vector.tensor_scalar_mul(
                out=out_sb[:, h * DH:(h + 1) * DH],
                in0=pot4[t][:, hh * (DH + 1):hh * (DH + 1) + DH],
                scalar1=rec[:, hh:hh + 1])

    nc.sync.dma_start(out=out, in_=out_sb)
    ctx.close()


def build_program():
    nc = bacc.Bacc(
        "TRN2", target_bir_lowering=False, debug=False,
        num_devices=NCORES)
    ins = {
        "pos": nc.dram_tensor("pos", [Q, L, D], FP32, kind="ExternalInput").ap(),
        "key": nc.dram_tensor("key", [L, D], FP32, kind="ExternalInput").ap(),
        "value": nc.dram_tensor("value", [L, D], FP32, kind="ExternalInput").ap(),
        "query": nc.dram_tensor("query", [Q, D], FP32, kind="ExternalInput").ap(),
        "mask": nc.dram_tensor("mask", [L], FP32, kind="ExternalInput").ap(),
        "Wk": nc.dram_tensor("Wk", [D, D], FP32, kind="ExternalInput").ap(),
        "Wq": nc.dram_tensor("Wq", [D, D], FP32, kind="ExternalInput").ap(),
        "Wv": nc.dram_tensor("Wv", [D, D], FP32, kind="ExternalInput").ap(),
        "Wr": nc.dram_tensor("Wr", [D, D], FP32, kind="ExternalInput").ap(),
        "bk": nc.dram_tensor("bk", [D], FP32, kind="ExternalInput").ap(),
        "bq": nc.dram_tensor("bq", [D], FP32, kind="ExternalInput").ap(),
        "bv": nc.dram_tensor("bv", [D], FP32, kind="ExternalInput").ap(),
        "u": nc.dram_tensor("u", [H, DH], FP32, kind="ExternalInput").ap(),
        "v": nc.dram_tensor("v", [H, DH], FP32, kind="ExternalInput").ap(),
    }
    outs = {
        "out": nc.dram_tensor("out", [Q, D], FP32, kind="ExternalOutput").ap(),
    }
    with tile.TileContext(nc) as tc:
        build_kernel_body(tc, outs, ins)
    nc.compile()
    return nc


def shard_inputs(inputs):
    """Full inputs -> list of 8 per-core input dicts (numpy, contiguous)."""
    f32 = lambda a: np.ascontiguousarray(np.asarray(a), dtype=np.float32)
    pos = f32(inputs["pos"])
    key = f32(inputs["key"])
    query = f32(inputs["query"])
    value = f32(inputs["value"])
    mask = f32(inputs["key_mask"])
    shared = {
        "Wk": f32(inputs["Wk"]), "Wq": f32(inputs["Wq"]),
        "Wv": f32(inputs["Wv"]), "Wr": f32(inputs["Wr"]),
        "bk": f32(inputs["bk"]), "bq": f32(inputs["bq"]),
        "bv": f32(inputs["bv"]),
        "u": f32(inputs["u"]), "v": f32(inputs["v"]),
    }
    in_maps = []
    for c in range(NCORES):
        b, q0 = c // 4, (c % 4) * Q
        m = dict(shared)
        m["pos"] = np.ascontiguousarray(pos[b, q0:q0 + Q])
        m["key"] = key[b]
        m["value"] = value[b]
        m["query"] = np.ascontiguousarray(query[b, q0:q0 + Q])
        m["mask"] = mask[b]
        in_maps.append(m)
    return in_maps


_CACHED = {}


def kernel(**inputs):
    from concourse.bass_utils import run_bass_kernel_spmd

    if "nc" not in _CACHED:
        _CACHED["nc"] = build_program()
    nc = _CACHED["nc"]
    in_maps = shard_inputs(inputs)
    res = run_bass_kernel_spmd(nc, in_maps, core_ids=list(range(NCORES)))
    out = np.zeros((B, L, D), dtype=np.float32)
    for c in range(NCORES):
        b, q0 = c // 4, (c % 4) * Q
        out[b, q0:q0 + Q] = res.results[c]["out"]
    return out
